# revision 7
# baseline (speedup 1.0000x reference)
"""GATv2 (2-layer, 8-head) message-passing kernel for Trainium2, 8 NeuronCores.

v2: batched SWDGE dma_gather replaces per-column indirect DMA.

Strategy (sharding_hint: partition edges by destination, nodes by range):
- Host: nodes split across 8 cores; per core, nodes are packed into 98 groups
  of 128 destination nodes via greedy bin-packing that minimizes the summed
  per-chunk max in-degree (the gather is split into 4 source-range chunks
  because dma_gather indices are int16, table has 100352 rows). Groups are
  coalesced into "runs" sharing quantized per-chunk widths so one dma_gather
  per (run, chunk) fetches all payload rows for several groups at once.
- Device, per core: project x -> h (PE); per layer compute xl/xr for owned
  nodes (PE), AllGather the xl table, expand it into a 256B-stride gather
  table, write 4 per-chunk poison rows (pad slots gather them; their logits
  land <= -6e3 so exp()==0), then per run: 4 dma_gathers + a short DVE chain
  (s=xl+xr, leaky-relu, *att, reduce-C, exp on ACT, softmax-normalize,
  weighted aggregation with free-dim reduces).
"""

import math
from dataclasses import dataclass

import numpy as np

# ---- problem constants (hardcoded; harness calls kernel(**inputs) directly) --
N = 100000
E = 3200000
IN_C = 1433
DIM = 32
HEADS = 8
OUT_C = 4
NUM_LAYER = 2
NUM_CLASS = 7
NEG_SLOPE = 0.2
NCORES = 8
BIG = 3.0e4
NCHUNK = 4
COLS_MAX = 208  # max gather-payload columns per run batch (SBUF budget)


@dataclass
class Cfg:
    ncores: int
    n_real: int
    nl: int           # owned (padded) nodes per core, = ngrp*128
    ngrp: int
    inp: int          # padded input feature dim (multiple of 128, >= IN_C+1)
    in_c: int
    d: int            # DIM
    h: int            # HEADS
    c: int            # OUT_C
    ncls: int
    nlayer: int
    npad: int
    ch: int           # chunk size (npad // 4)
    runs: tuple       # per run: (g0, R, (w0,w1,w2,w3), colbase)
    sw: int           # total payload columns (sum over runs of R*sum(w))
    wcmax: int        # max chunk width
    debug: bool = False


def _ceil_to(x, m):
    return (x + m - 1) // m * m


def _greedy_pack(prof_k, ngrp):
    """Pack len(prof_k) nodes (rows of [4] chunk profiles) into ngrp groups of
    <=128 minimizing sum_g sum_c max. Returns (assign, gmax)."""
    n = prof_k.shape[0]
    order = np.argsort(-prof_k.max(axis=1), kind="stable")
    gmax = np.zeros((ngrp, NCHUNK), np.int64)
    gcnt = np.zeros(ngrp, np.int64)
    assign = np.empty(n, np.int64)
    for i in order:
        pi = prof_k[i]
        inc = np.maximum(gmax, pi[None, :]).sum(axis=1) - gmax.sum(axis=1)
        score = inc * 1000 - gcnt
        score[gcnt >= 128] = 1 << 50
        g = int(np.argmin(score))
        assign[i] = g
        gmax[g] = np.maximum(gmax[g], pi)
        gcnt[g] += 1
    return assign, gmax


def host_prep(x, edge_index, w_proj, b_proj, w_l, b_l, w_r, b_r, att, conv_bias,
              w_pred, b_pred, ncores):
    """Numpy preprocessing: node->core/group assignment, chunked gather grid,
    int16 index arrays, padded/transposed inputs."""
    x = np.asarray(x)
    edge_index = np.asarray(edge_index)
    n_real, in_c = x.shape
    d = w_proj.shape[1]
    h, c = att.shape[1], att.shape[2]
    nlayer = w_l.shape[0]
    ncls = w_pred.shape[1]

    nl = _ceil_to(_ceil_to(n_real, ncores) // ncores, 128)
    ngrp = nl // 128
    npad = nl * ncores
    ch = npad // NCHUNK
    assert npad % NCHUNK == 0 and ch <= 32768
    nfake = npad - n_real
    assert nfake >= NCHUNK - 1

    # node -> core: contiguous real ranges; cores 1,3,5 donate one slot to a
    # fake node (their last table row becomes that chunk's poison row), core 7
    # takes the rest of the fakes (incl. chunk 3's poison row).
    fakes_per_core = [0, 1, 0, 1, 0, 1, 0, nfake - 3]
    real_counts = [nl - f for f in fakes_per_core]
    assert sum(real_counts) == n_real
    bounds = np.concatenate([[0], np.cumsum(real_counts)])
    node_core = np.empty(n_real, np.int64)
    for k in range(ncores):
        node_core[bounds[k]:bounds[k + 1]] = k

    src = edge_index[0].astype(np.int64)
    dst = edge_index[1].astype(np.int64)
    src_chunk = node_core[src] // 2
    prof = np.zeros((n_real, NCHUNK), np.int32)
    np.add.at(prof, (dst, src_chunk), 1)
    assert prof.sum(axis=1).min() >= 1, "zero-degree real node"

    # pad poison must dominate logits
    att_flat = att.reshape(nlayer, h * c)
    min_att_sum = np.abs(att_flat).reshape(nlayer, h, c).sum(-1).min()
    assert min_att_sum * NEG_SLOPE * BIG > 30, f"pad poison too weak: {min_att_sum}"

    # per-core packing
    nodes_by_lid = np.empty((ncores, nl), np.int64)
    gmax_all = np.zeros((ncores, ngrp, NCHUNK), np.int64)
    fake_ids = iter(range(n_real, n_real + nfake))
    for k in range(ncores):
        ids_k = np.arange(bounds[k], bounds[k + 1])
        assign, gmax = _greedy_pack(prof[ids_k], ngrp)
        # sort groups by width-sum desc (cross-core alignment)
        gorder = np.argsort(-gmax.sum(axis=1), kind="stable")
        rank_of = np.empty(ngrp, np.int64)
        rank_of[gorder] = np.arange(ngrp)
        groups = [[] for _ in range(ngrp)]
        for i, g in enumerate(assign):
            groups[rank_of[g]].append(ids_k[i])
        gmax = gmax[gorder]
        nf = fakes_per_core[k]
        if nf > 0 and len(groups[-1]) == 128:
            # make room in the last group: move one member to a deficit group
            j = next(jj for jj in range(ngrp) if len(groups[jj]) < 128)
            mv = groups[-1].pop()
            groups[j].append(mv)
            gmax[j] = np.maximum(gmax[j], prof[mv])
        # fill deficits with fakes, last group last (poison row = last slot)
        for g in range(ngrp - 1, -1, -1):
            while len(groups[g]) < 128:
                groups[g].append(next(fake_ids))
        nodes_by_lid[k] = np.concatenate([np.asarray(g, np.int64) for g in groups])
        gmax_all[k] = gmax
        if nf > 0:
            assert nodes_by_lid[k, nl - 1] >= n_real, "poison row must be fake"

    wgc = gmax_all.max(axis=0)  # [ngrp, 4] cross-core widths

    # runs: contiguous group ranks sharing quantized widths
    runs = []
    g0 = 0
    colbase = 0
    while g0 < ngrp:
        R = 1
        rw = wgc[g0].copy()
        while g0 + R < ngrp:
            rw2 = np.maximum(rw, wgc[g0 + R])
            if (R + 1) * rw2.sum() > COLS_MAX:
                break
            rw = rw2
            R += 1
        runs.append((g0, R, tuple(int(w) for w in rw), colbase))
        colbase += R * int(rw.sum())
        g0 += R
    sw = colbase
    wcmax = max(max(r[2]) for r in runs)

    # table positions
    tab_pos = np.empty(npad, np.int64)
    for k in range(ncores):
        tab_pos[nodes_by_lid[k]] = k * nl + np.arange(nl)

    # per-edge slot assignment
    run_of_g = np.empty(ngrp, np.int64)
    for ri, (rg0, R, rw, cb) in enumerate(runs):
        run_of_g[rg0:rg0 + R] = ri
    run_arr = np.array([(rg0, R, cb) for (rg0, R, rw, cb) in runs], np.int64)
    rw_arr = np.array([r[2] for r in runs], np.int64)              # [nrun, 4]
    rcum = np.concatenate([np.zeros((len(runs), 1), np.int64),
                           np.cumsum(rw_arr, axis=1)], axis=1)     # [nrun, 5]

    k_dst = node_core[dst]
    lid = tab_pos[dst] - k_dst * nl
    p_e = lid % 128
    g_e = lid // 128
    ri_e = run_of_g[g_e]
    # j = rank of edge within (dst, chunk)
    eo = np.lexsort((src_chunk, dst))
    ds, cs = dst[eo], src_chunk[eo]
    key = ds * NCHUNK + cs
    _, starts, counts = np.unique(key, return_index=True, return_counts=True)
    j_s = np.arange(len(eo)) - np.repeat(starts, counts)
    j_e = np.empty(len(eo), np.int64)
    j_e[eo] = j_s
    assert (j_e < rw_arr[ri_e, src_chunk]).all()
    col = (run_arr[ri_e, 2]                                  # run colbase
           + (g_e - run_arr[ri_e, 0]) * rw_arr[ri_e, src_chunk]
           + rcum[ri_e, src_chunk] * run_arr[ri_e, 1]        # region start
           + j_e)
    val = (tab_pos[src] - src_chunk * ch).astype(np.int16)
    assert (tab_pos[src] // ch == src_chunk).all()

    # int16 index arrays: slot (p, col) -> idx16[p%16, 8*col + p//16]
    idx16 = np.full((ncores, 16, 8 * sw), ch - 1, np.int16)  # default: poison
    idx16[k_dst, p_e % 16, 8 * col + p_e // 16] = val

    # x, transposed + padded, with ones row for the bias trick
    inp = _ceil_to(in_c + 1, 128)
    per_core = []
    for k in range(ncores):
        xt = np.zeros((inp, nl), np.float32)
        nodes = nodes_by_lid[k]
        real = nodes < n_real
        xt[:in_c, real] = x[nodes[real]].T
        xt[in_c, :] = 1.0
        per_core.append({"x_t": xt,
                         "idx_all": np.ascontiguousarray(np.tile(idx16[k], (8, 1)))})

    wp_pad = np.zeros((inp, d), np.float32)
    wp_pad[:in_c] = w_proj
    wp_pad[in_c] = b_proj
    shared = {"w_proj": wp_pad,
              "w_pred": np.vstack([w_pred, b_pred[None, :]]).astype(np.float32)}
    for l in range(nlayer):
        shared[f"wl{l}"] = np.vstack([w_l[l], b_l[l][None, :]]).astype(np.float32)
        shared[f"wr{l}"] = np.vstack([w_r[l], b_r[l][None, :]]).astype(np.float32)
        shared[f"att{l}"] = np.broadcast_to(
            att_flat[l][None, None, :], (128, wcmax, h * c)
        ).reshape(128, wcmax * h * c).astype(np.float32)
        shared[f"cb{l}"] = np.broadcast_to(conv_bias[l][None, :], (128, h * c)).astype(np.float32)
        shared[f"padv{l}"] = np.broadcast_to(
            (-np.sign(att_flat[l]) * BIG).astype(np.float32)[None, :], (NCHUNK, h * c)
        ).copy()

    cfg = Cfg(ncores=ncores, n_real=n_real, nl=nl, ngrp=ngrp, inp=inp, in_c=in_c,
              d=d, h=h, c=c, ncls=ncls, nlayer=nlayer, npad=npad, ch=ch,
              runs=tuple(runs), sw=sw, wcmax=wcmax)
    meta = {"nodes_by_lid": nodes_by_lid}
    return cfg, per_core, shared, meta


def build_program(cfg: Cfg):
    import concourse.bass as bass
    import concourse.bacc as bacc
    import concourse.mybir as mybir
    import concourse.tile as tile
    from concourse.masks import make_identity
    from concourse.tile import add_dep_helper

    f32 = mybir.dt.float32
    i16 = mybir.dt.int16
    P = 128
    D, H, C = cfg.d, cfg.h, cfg.c
    NGRP, NL, CH = cfg.ngrp, cfg.nl, cfg.ch
    HS = D + 1  # h chunk stride (extra ones column for the bias-row trick)
    EW = 2 * D  # gather row width in f32 (256B)

    nc = bacc.Bacc(trn_type="TRN2", num_devices=cfg.ncores)

    x_t = nc.dram_tensor("x_t", [cfg.inp, NL], f32, kind="ExternalInput")
    idx_in = nc.dram_tensor("idx_all", [P, 8 * cfg.sw], i16, kind="ExternalInput")
    wp_in = nc.dram_tensor("w_proj", [cfg.inp, D], f32, kind="ExternalInput")
    wpred_in = nc.dram_tensor("w_pred", [D + 1, cfg.ncls], f32, kind="ExternalInput")
    wl_in = [nc.dram_tensor(f"wl{l}", [D + 1, D], f32, kind="ExternalInput") for l in range(cfg.nlayer)]
    wr_in = [nc.dram_tensor(f"wr{l}", [D + 1, D], f32, kind="ExternalInput") for l in range(cfg.nlayer)]
    att_in = [nc.dram_tensor(f"att{l}", [P, cfg.wcmax * D], f32, kind="ExternalInput") for l in range(cfg.nlayer)]
    cb_in = [nc.dram_tensor(f"cb{l}", [P, D], f32, kind="ExternalInput") for l in range(cfg.nlayer)]
    padv_in = [nc.dram_tensor(f"padv{l}", [NCHUNK, D], f32, kind="ExternalInput") for l in range(cfg.nlayer)]
    out_dram = nc.dram_tensor("out", [P, NGRP * cfg.ncls], f32, kind="ExternalOutput")
    dbg_h = (nc.dram_tensor("dbg_h", [P, NGRP * (D + 1)], f32, kind="ExternalOutput")
             if cfg.debug else None)

    xl_own = nc.dram_tensor("xl_own", [NL, D], f32)
    if cfg.ncores > 4:  # shared-output collectives need >4 cores
        xl_allg = nc.dram_tensor("xl_allg", [cfg.npad, D], f32, addr_space="Shared")
    else:
        xl_allg = nc.dram_tensor("xl_allg", [cfg.npad, D], f32)
    # gather table: 256B rows, one spare row so the last row's 256B read is safe
    xl_tab = nc.dram_tensor("xl_tab", [cfg.npad + 1, EW], f32)

    with tile.TileContext(nc) as tc:
        with (
            tc.tile_pool(name="const", bufs=1) as cp,
            tc.tile_pool(name="pers", bufs=1) as pp,
            tc.tile_pool(name="work", bufs=3) as wp,
            tc.tile_pool(name="gath", bufs=2) as gp,
            tc.tile_pool(name="edge", bufs=1) as ep,
            tc.tile_pool(name="small", bufs=2) as sp,
            tc.tile_pool(name="ps_mm", bufs=2, space="PSUM") as pmm,
            tc.tile_pool(name="ps_tr", bufs=2, space="PSUM") as ptr,
        ):
            # ---- constants -> SBUF ----
            ident = cp.tile([P, P], f32)
            make_identity(nc, ident[:])
            nj = cfg.inp // P
            wp_sb = cp.tile([P, nj * D], f32)
            nc.sync.dma_start(out=wp_sb[:].rearrange("p (j d) -> p j d", d=D),
                              in_=wp_in[:].rearrange("(j p) d -> p j d", p=P))
            wl_sb = [cp.tile([D + 1, D], f32, name=f"wl_sb{l}") for l in range(cfg.nlayer)]
            wr_sb = [cp.tile([D + 1, D], f32, name=f"wr_sb{l}") for l in range(cfg.nlayer)]
            cb_sb = [cp.tile([P, D], f32, name=f"cb_sb{l}") for l in range(cfg.nlayer)]
            for l in range(cfg.nlayer):
                nc.sync.dma_start(out=wl_sb[l][:], in_=wl_in[l][:])
                nc.sync.dma_start(out=wr_sb[l][:], in_=wr_in[l][:])
                nc.sync.dma_start(out=cb_sb[l][:], in_=cb_in[l][:])
            wpred_sb = cp.tile([D + 1, cfg.ncls], f32)
            nc.sync.dma_start(out=wpred_sb[:], in_=wpred_in[:])
            att_sb = pp.tile([P, cfg.wcmax * D], f32)   # reloaded per layer
            ones_sb = cp.tile([P, 1], f32)
            nc.gpsimd.memset(ones_sb[:], 1.0)

            h_a = pp.tile([P, NGRP * HS], f32, name="h_a")
            h_b = pp.tile([P, NGRP * HS], f32, name="h_b")
            xr_own = pp.tile([P, NGRP * D], f32)

            def h_view(t):  # [P, NGRP, D] data columns
                return t[:].rearrange("p (g s) -> p g s", s=HS)[:, :, :D]

            def ones_col(t):
                return t[:].rearrange("p (g s) -> p g s", s=HS)[:, :, D:HS]

            # ---- P1: h0 = x @ w_proj + b_proj ----
            col_tiles = []
            c0 = 0
            while c0 < NL:
                tw = min(512, NL - c0)
                col_tiles.append((c0, tw))
                c0 += tw
            for (c0, tw) in col_tiles:
                h_acc = pmm.tile([D, 512], f32, tag="h_acc")
                for jj in range(nj):
                    xtile = wp.tile([P, 512], f32, tag="xtile")
                    nc.sync.dma_start(out=xtile[:, :tw], in_=x_t[jj * P:(jj + 1) * P, c0:c0 + tw])
                    nc.tensor.matmul(out=h_acc[:, :tw], lhsT=wp_sb[:, jj * D:(jj + 1) * D],
                                     rhs=xtile[:, :tw], start=(jj == 0), stop=(jj == nj - 1))
                hT_stage = wp.tile([D, 512], f32, tag="hT_stage")
                nc.scalar.copy(out=hT_stage[:, :tw], in_=h_acc[:, :tw])
                for t2 in range(tw // P):
                    chk = (c0 + t2 * P) // P
                    htr = ptr.tile([P, D], f32, tag="htr", bufs=1)
                    nc.tensor.transpose(out=htr[:], in_=hT_stage[:, t2 * P:(t2 + 1) * P],
                                        identity=ident[:D, :D])
                    nc.vector.tensor_copy(out=h_view(h_a)[:, chk, :], in_=htr[:])
            nc.vector.tensor_copy(out=ones_col(h_a)[:, :, 0], in_=ones_sb[:].to_broadcast([P, NGRP]))

            h_cur, h_nxt = h_a, h_b

            # ---- P2: layers ----
            for l in range(cfg.nlayer):
                nc.sync.dma_start(out=att_sb[:], in_=att_in[l][:])
                # (a) xl/xr for owned nodes; xl -> DRAM (+allgather), xr -> SBUF
                xl_dmas = []
                nbatch = math.ceil(NGRP / 16)
                for b in range(nbatch):
                    chunks = range(b * 16, min((b + 1) * 16, NGRP))
                    hT_chs = {}
                    for chk in chunks:
                        tr = ptr.tile([HS, P], f32, tag="tr")
                        nc.tensor.transpose(
                            out=tr[:], in_=h_cur[:].rearrange("p (g s) -> p g s", s=HS)[:, chk, :],
                            identity=ident[:])
                        hT_ch = wp.tile([HS, P], f32, tag="hT_ch")
                        nc.scalar.copy(out=hT_ch[:], in_=tr[:])
                        hT_chs[chk] = hT_ch
                    for (dst_sb, w_t, to_dram) in ((None, wl_sb[l], True), (xr_own, wr_sb[l], False)):
                        big = pmm.tile([P, 512], f32, tag="big")
                        for i, chk in enumerate(chunks):
                            nc.tensor.matmul(out=big[:, i * D:(i + 1) * D], lhsT=hT_chs[chk][:],
                                             rhs=w_t[:], start=True, stop=True)
                        ncols = len(chunks) * D
                        if to_dram:
                            stage = wp.tile([P, 512], f32, tag="xl_stage")
                            nc.scalar.copy(out=stage[:, :ncols], in_=big[:, :ncols])
                            dma = nc.sync.dma_start(
                                out=xl_own[:].rearrange("(a p) d -> p a d", p=P)[
                                    :, b * 16:b * 16 + len(chunks), :],
                                in_=stage[:, :ncols].rearrange("p (a d) -> p a d", d=D))
                            xl_dmas.append(dma)
                        else:
                            nc.scalar.copy(out=dst_sb[:, b * 16 * D: b * 16 * D + ncols],
                                           in_=big[:, :ncols])
                # (b) allgather xl; expand into 256B-stride table; poison rows
                if cfg.ncores > 1:
                    cc = nc.gpsimd.collective_compute(
                        "AllGather", mybir.AluOpType.bypass,
                        replica_groups=[list(range(cfg.ncores))],
                        ins=[xl_own[:]], outs=[xl_allg[:]])
                    for dma in xl_dmas:
                        add_dep_helper(cc.ins, dma.ins)
                    cpls = []
                    for ci in range(NCHUNK):
                        cpl = nc.sync.dma_start(
                            out=xl_tab[ci * CH:(ci + 1) * CH, :D],
                            in_=xl_allg[ci * CH:(ci + 1) * CH, :])
                        add_dep_helper(cpl.ins, cc.ins)
                        cpls.append(cpl)
                else:
                    cpls = []
                    for ci in range(NCHUNK):
                        lo, hi = ci * CH, min((ci + 1) * CH, NL)
                        if lo >= hi:
                            continue
                        cpl = nc.sync.dma_start(out=xl_tab[lo:hi, :D],
                                                in_=xl_own[lo:hi, :])
                        for dma in xl_dmas:
                            add_dep_helper(cpl.ins, dma.ins)
                        cpls.append(cpl)
                pv = nc.sync.dma_start(
                    out=xl_tab[0:cfg.npad, :].rearrange("(a r) e -> a r e", r=CH)[:, CH - 1, :D],
                    in_=padv_in[l][:])
                for cpl in cpls:
                    add_dep_helper(pv.ins, cpl.ins)

                # (c) edge phase per run
                for (g0, R, rw, colbase) in cfg.runs:
                    cols = R * sum(rw)
                    rcum = [0]
                    for w in rw:
                        rcum.append(rcum[-1] + w)
                    idx_sb = gp.tile([P, 8 * COLS_MAX], i16, tag="idx")
                    nc.sync.dma_start(out=idx_sb[:, :8 * cols],
                                      in_=idx_in[:, 8 * colbase: 8 * (colbase + cols)])
                    xg = gp.tile([P, COLS_MAX * EW], f32, tag="xg")
                    xgq = xg[:].rearrange("p (q e) -> p q e", e=EW)
                    for cidx in range(NCHUNK):
                        W = rw[cidx]
                        if W == 0:
                            continue
                        rs = R * rcum[cidx]
                        ncol = R * W
                        # dma_gather is only reliable up to 1024 idxs/call
                        # (HW-probed); split into <=8-column calls.
                        for off in range(0, ncol, 8):
                            nn = min(8, ncol - off)
                            gth = nc.gpsimd.dma_gather(
                                out_ap=xgq[:, rs + off:rs + off + nn, :],
                                in_ap=xl_tab[cidx * CH:(cidx + 1) * CH, :],
                                idxs_ap=idx_sb[:, 8 * (rs + off): 8 * (rs + off + nn)],
                                num_idxs=128 * nn,
                                num_idxs_reg=128 * nn,
                                elem_size=EW,
                            )
                            add_dep_helper(gth.ins, pv.ins)
                    ex = ep.tile([P, COLS_MAX * H], f32, tag="ex")
                    exq = ex[:].rearrange("p (q h) -> p q h", h=H)
                    # per chunk-region: s = xl + xr (into upper half), leaky,
                    # *att, logit reduce
                    for cidx in range(NCHUNK):
                        W = rw[cidx]
                        if W == 0:
                            continue
                        rs = R * rcum[cidx]
                        ncol = R * W
                        reg = xgq[:, rs:rs + ncol, :]
                        reg_r = reg.rearrange("p (r w) e -> p r w e", w=W)
                        nc.vector.tensor_tensor(
                            out=reg_r[:, :, :, D:EW],
                            in0=reg_r[:, :, :, 0:D],
                            in1=xr_own[:].rearrange("p (g d) -> p g d", d=D)[
                                :, g0:g0 + R, :].unsqueeze(2).to_broadcast([P, R, W, D]),
                            op=mybir.AluOpType.add)
                        s_flat = reg[:, :, D:EW]
                        nc.vector.scalar_tensor_tensor(
                            out=s_flat, in0=s_flat, scalar=NEG_SLOPE, in1=s_flat,
                            op0=mybir.AluOpType.mult, op1=mybir.AluOpType.max)
                        nc.vector.tensor_tensor(
                            out=reg_r[:, :, :, D:EW], in0=reg_r[:, :, :, D:EW],
                            in1=att_sb[:].rearrange("p (w d) -> p w d", d=D)[
                                :, :W, :].unsqueeze(1).to_broadcast([P, R, W, D]),
                            op=mybir.AluOpType.mult)
                        nc.vector.tensor_reduce(
                            out=exq[:, rs:rs + ncol, :],
                            in_=reg[:, :, D:EW].rearrange("p q (hh cc) -> p q hh cc", cc=C),
                            axis=mybir.AxisListType.X, op=mybir.AluOpType.add)
                        nc.scalar.activation(
                            out=ex[:, rs * H:(rs + ncol) * H],
                            in_=ex[:, rs * H:(rs + ncol) * H],
                            func=mybir.ActivationFunctionType.Exp)
                    # denominators: per chunk-region reduce over W, then sum
                    den4 = sp.tile([P, NCHUNK * R * H], f32, tag="den4")
                    nreg = 0
                    for cidx in range(NCHUNK):
                        W = rw[cidx]
                        if W == 0:
                            continue
                        rs = R * rcum[cidx]
                        nc.vector.tensor_reduce(
                            out=den4[:, nreg * R * H:(nreg + 1) * R * H],
                            in_=exq[:, rs:rs + R * W, :].rearrange(
                                "p (r w) h -> p r h w", w=W),
                            axis=mybir.AxisListType.X, op=mybir.AluOpType.add)
                        nreg += 1
                    den = sp.tile([P, R * H], f32, tag="den")
                    nc.vector.tensor_reduce(
                        out=den[:],
                        in_=den4[:, :nreg * R * H].rearrange(
                            "p (n q) -> p q n", n=nreg),
                        axis=mybir.AxisListType.X, op=mybir.AluOpType.add)
                    nc.vector.tensor_scalar_max(den[:], den[:], 1e-16)
                    rec = sp.tile([P, R * H], f32, tag="rec")
                    nc.vector.reciprocal(out=rec[:], in_=den[:])
                    # weighted aggregation: tmp = xl * ex (in place over xl),
                    # numer = sum_w tmp
                    num4 = sp.tile([P, NCHUNK * R * D], f32, tag="num4")
                    nreg = 0
                    for cidx in range(NCHUNK):
                        W = rw[cidx]
                        if W == 0:
                            continue
                        rs = R * rcum[cidx]
                        ncol = R * W
                        reg = xgq[:, rs:rs + ncol, :]
                        nc.vector.tensor_tensor(
                            out=reg[:, :, 0:D].rearrange("p q (hh cc) -> p q hh cc", cc=C),
                            in0=reg[:, :, 0:D].rearrange("p q (hh cc) -> p q hh cc", cc=C),
                            in1=exq[:, rs:rs + ncol, :].unsqueeze(3).to_broadcast(
                                [P, ncol, H, C]),
                            op=mybir.AluOpType.mult)
                        nc.vector.tensor_reduce(
                            out=num4[:, nreg * R * D:(nreg + 1) * R * D],
                            in_=reg[:, :, 0:D].rearrange("p (r w) d -> p r d w", w=W),
                            axis=mybir.AxisListType.X, op=mybir.AluOpType.add)
                        nreg += 1
                    numer = sp.tile([P, R * D], f32, tag="numer")
                    nc.vector.tensor_reduce(
                        out=numer[:],
                        in_=num4[:, :nreg * R * D].rearrange(
                            "p (n q) -> p q n", n=nreg),
                        axis=mybir.AxisListType.X, op=mybir.AluOpType.add)
                    o1 = sp.tile([P, R * D], f32, tag="o1")
                    nc.vector.tensor_tensor(
                        out=o1[:].rearrange("p (r hh cc) -> p r hh cc", r=R, cc=C),
                        in0=numer[:].rearrange("p (r hh cc) -> p r hh cc", r=R, cc=C),
                        in1=rec[:].rearrange("p (r hh) -> p r hh", r=R).unsqueeze(
                            3).to_broadcast([P, R, H, C]),
                        op=mybir.AluOpType.mult)
                    o3 = sp.tile([P, R * D], f32, tag="o3")
                    nc.vector.tensor_tensor(
                        out=o3[:].rearrange("p (r d) -> p r d", d=D),
                        in0=o1[:].rearrange("p (r d) -> p r d", d=D),
                        in1=h_view(h_cur)[:, g0:g0 + R, :],
                        op=mybir.AluOpType.add)
                    nc.vector.tensor_tensor(
                        out=h_view(h_nxt)[:, g0:g0 + R, :],
                        in0=o3[:].rearrange("p (r d) -> p r d", d=D),
                        in1=cb_sb[l][:].unsqueeze(1).to_broadcast([P, R, D]),
                        op=mybir.AluOpType.add)
                nc.vector.tensor_copy(out=ones_col(h_nxt)[:, :, 0],
                                      in_=ones_sb[:].to_broadcast([P, NGRP]))
                h_cur, h_nxt = h_nxt, h_cur

            if dbg_h is not None:
                nc.sync.dma_start(out=dbg_h[:], in_=h_cur[:])

            # ---- P3: out = h2 @ w_pred + b_pred ----
            nbatch = math.ceil(NGRP / 16)
            for b in range(nbatch):
                chunks = range(b * 16, min((b + 1) * 16, NGRP))
                fin = pmm.tile([P, 16 * cfg.ncls], f32, tag="fin", bufs=1)
                for i, chk in enumerate(chunks):
                    tr = ptr.tile([HS, P], f32, tag="tr")
                    nc.tensor.transpose(
                        out=tr[:], in_=h_cur[:].rearrange("p (g s) -> p g s", s=HS)[:, chk, :],
                        identity=ident[:])
                    hT_ch = wp.tile([HS, P], f32, tag="hT_ch")
                    nc.scalar.copy(out=hT_ch[:], in_=tr[:])
                    nc.tensor.matmul(out=fin[:, i * cfg.ncls:(i + 1) * cfg.ncls], lhsT=hT_ch[:],
                                     rhs=wpred_sb[:], start=True, stop=True)
                ncols = len(chunks) * cfg.ncls
                ostage = wp.tile([P, 16 * cfg.ncls], f32, tag="ostage")
                nc.scalar.copy(out=ostage[:, :ncols], in_=fin[:, :ncols])
                nc.sync.dma_start(out=out_dram[:, b * 16 * cfg.ncls: b * 16 * cfg.ncls + ncols],
                                  in_=ostage[:, :ncols])
    nc.finalize()
    return nc


def assemble_output(cfg: Cfg, meta, core_outs):
    """core_outs: list of [128, NGRP*ncls] arrays -> full [n_real, ncls]."""
    full = np.zeros((cfg.npad, cfg.ncls), np.float32)
    for k in range(cfg.ncores):
        o = core_outs[k].reshape(128, cfg.ngrp, cfg.ncls).transpose(1, 0, 2).reshape(cfg.nl, cfg.ncls)
        full[meta["nodes_by_lid"][k]] = o
    return full[:cfg.n_real]


_LAST = {}


def bench(inputs, iters=20) -> dict:
    """Correctness + repeat-execution timing via a hand-rolled PJRT runner
    (no NTFF hook in this environment). Returns output + per-iter seconds."""
    import time

    import jax
    from jax.sharding import Mesh, NamedSharding, PartitionSpec
    from jax.experimental.shard_map import shard_map
    import concourse.mybir as mybir
    from concourse import bass2jax
    from concourse.bass2jax import _bass_exec_p, install_neuronx_cc_hook, partition_id_tensor

    inputs = {k: np.asarray(v) for k, v in inputs.items()}
    cfg, per_core, shared, meta = host_prep(ncores=NCORES, **inputs)
    nc = build_program(cfg)
    in_maps = [{**shared, **pc} for pc in per_core]

    install_neuronx_cc_hook()
    partition_name = nc.partition_id_tensor.name if nc.partition_id_tensor else None
    in_names, out_names, out_avals, zero_outs = [], [], [], []
    for alloc in nc.m.functions[0].allocations:
        if not isinstance(alloc, mybir.MemoryLocationSet):
            continue
        name = alloc.memorylocations[0].name
        if alloc.kind == "ExternalInput":
            if name != partition_name:
                in_names.append(name)
        elif alloc.kind == "ExternalOutput":
            shape = tuple(alloc.tensor_shape)
            dtype = mybir.dt.np(alloc.dtype)
            out_names.append(name)
            out_avals.append(jax.core.ShapedArray(shape, dtype))
            zero_outs.append(np.zeros(shape, dtype))
    n_params = len(in_names)
    n_outs = len(out_avals)
    all_in_names = list(in_names) + list(out_names)
    if partition_name is not None:
        all_in_names.append(partition_name)
    donate = tuple(range(n_params, n_params + n_outs))

    def _body(*args):
        operands = list(args)
        if partition_name is not None:
            operands.append(partition_id_tensor())
        return tuple(_bass_exec_p.bind(
            *operands, out_avals=tuple(out_avals), in_names=tuple(all_in_names),
            out_names=tuple(out_names), lowering_input_output_aliases=(),
            sim_require_finite=True, sim_require_nnan=True, nc=nc))

    devices = jax.devices()[:NCORES]
    mesh = Mesh(np.asarray(devices), ("core",))
    in_specs = (PartitionSpec("core"),) * (n_params + n_outs)
    out_specs = (PartitionSpec("core"),) * n_outs
    sharded = jax.jit(shard_map(_body, mesh=mesh, in_specs=in_specs,
                                out_specs=out_specs, check_rep=False),
                      donate_argnums=donate, keep_unused=True)
    concat_in = [np.concatenate([np.asarray(in_maps[c][n]) for c in range(NCORES)], axis=0)
                 for n in in_names]
    t0 = time.time()
    sh = NamedSharding(mesh, PartitionSpec("core"))
    args_dev = [jax.device_put(a, sh) for a in concat_in]
    jax.block_until_ready(args_dev)
    t_put = time.time() - t0

    def zeros_dev():
        return [jax.device_put(np.zeros((NCORES * z.shape[0], *z.shape[1:]), z.dtype), sh)
                for z in zero_outs]

    t0 = time.time()
    outs = sharded(*args_dev, *zeros_dev())
    jax.block_until_ready(outs)
    t_first = time.time() - t0
    result = [
        {name: np.asarray(outs[i]).reshape(NCORES, *out_avals[i].shape)[c]
         for i, name in enumerate(out_names)} for c in range(NCORES)]

    # warm + timed loop (zeros pre-staged on device; donated per call)
    zsets = [zeros_dev() for _ in range(iters + 2)]
    jax.block_until_ready(zsets)
    r = sharded(*args_dev, *zsets[0])
    jax.block_until_ready(r)
    r = sharded(*args_dev, *zsets[1])
    jax.block_until_ready(r)
    t0 = time.time()
    rs = [sharded(*args_dev, *zsets[2 + i]) for i in range(iters)]
    jax.block_until_ready(rs)
    per_iter = (time.time() - t0) / iters

    out_full = assemble_output(cfg, meta, [result[k]["out"] for k in range(NCORES)])
    return {"out": out_full, "per_iter_s": per_iter, "first_s": t_first,
            "put_s": t_put, "cfg": cfg}


def kernel(**inputs) -> np.ndarray:
    import time

    from concourse.bass_utils import run_bass_kernel_spmd

    inputs = {k: np.asarray(v) for k, v in inputs.items()}
    cfg, per_core, shared, meta = host_prep(ncores=NCORES, **inputs)
    nc = build_program(cfg)
    in_maps = [{**shared, **pc} for pc in per_core]
    res = None
    for attempt in range(2):
        try:
            res = run_bass_kernel_spmd(nc, in_maps, core_ids=list(range(NCORES)))
            break
        except Exception:
            # transient device wedge (NRT_EXEC_UNIT_UNRECOVERABLE) recovers on
            # a fresh attempt once the runtime re-initializes
            if attempt == 1:
                raise
            time.sleep(20)
    assert res is not None
    _LAST["res"] = res
    _LAST["cfg"] = cfg
    outs = [res.results[k]["out"] for k in range(NCORES)]
    return assemble_output(cfg, meta, outs)


# revision 15
# speedup vs baseline: 1.9908x; 1.9908x over previous
"""GATv2 (2-layer, 8-head) message-passing kernel for Trainium2, 8 NeuronCores.

v2: batched SWDGE dma_gather replaces per-column indirect DMA.

Strategy (sharding_hint: partition edges by destination, nodes by range):
- Host: nodes split across 8 cores; per core, nodes are packed into 98 groups
  of 128 destination nodes via greedy bin-packing that minimizes the summed
  per-chunk max in-degree (the gather is split into 4 source-range chunks
  because dma_gather indices are int16, table has 100352 rows). Groups are
  coalesced into "runs" sharing quantized per-chunk widths so one dma_gather
  per (run, chunk) fetches all payload rows for several groups at once.
- Device, per core: project x -> h (PE); per layer compute xl/xr for owned
  nodes (PE), AllGather the xl table, expand it into a 256B-stride gather
  table, write 4 per-chunk poison rows (pad slots gather them; their logits
  land <= -6e3 so exp()==0), then per run: 4 dma_gathers + a short DVE chain
  (s=xl+xr, leaky-relu, *att, reduce-C, exp on ACT, softmax-normalize,
  weighted aggregation with free-dim reduces).
"""

import math
from dataclasses import dataclass

import numpy as np

# ---- problem constants (hardcoded; harness calls kernel(**inputs) directly) --
N = 100000
E = 3200000
IN_C = 1433
DIM = 32
HEADS = 8
OUT_C = 4
NUM_LAYER = 2
NUM_CLASS = 7
NEG_SLOPE = 0.2
NCORES = 8
BIG = 3.0e4
NCHUNK = 4
COLS_MAX = 208  # max gather-payload columns per run batch (SBUF budget)


@dataclass
class Cfg:
    ncores: int
    n_real: int
    nl: int           # owned (padded) nodes per core, = ngrp*128
    ngrp: int
    inp: int          # padded input feature dim (multiple of 128, >= IN_C+1)
    in_c: int
    d: int            # DIM
    h: int            # HEADS
    c: int            # OUT_C
    ncls: int
    nlayer: int
    npad: int
    ch: int           # chunk size (npad // 4)
    runs: tuple       # per run: (g0, R, (w0,w1,w2,w3), colbase)
    sw: int           # total payload columns (sum over runs of R*sum(w))
    wcmax: int        # max chunk width
    debug: bool = False


def _ceil_to(x, m):
    return (x + m - 1) // m * m


def _greedy_pack(prof_k, ngrp):
    """Pack len(prof_k) nodes (rows of [4] chunk profiles) into ngrp groups of
    <=128 minimizing sum_g sum_c max. Returns (assign, gmax)."""
    n = prof_k.shape[0]
    order = np.argsort(-prof_k.max(axis=1), kind="stable")
    gmax = np.zeros((ngrp, NCHUNK), np.int64)
    gcnt = np.zeros(ngrp, np.int64)
    assign = np.empty(n, np.int64)
    for i in order:
        pi = prof_k[i]
        inc = np.maximum(gmax, pi[None, :]).sum(axis=1) - gmax.sum(axis=1)
        score = inc * 1000 - gcnt
        score[gcnt >= 128] = 1 << 50
        g = int(np.argmin(score))
        assign[i] = g
        gmax[g] = np.maximum(gmax[g], pi)
        gcnt[g] += 1
    return assign, gmax


def host_prep(x, edge_index, w_proj, b_proj, w_l, b_l, w_r, b_r, att, conv_bias,
              w_pred, b_pred, ncores):
    """Numpy preprocessing: node->core/group assignment, chunked gather grid,
    int16 index arrays, padded/transposed inputs."""
    x = np.asarray(x)
    edge_index = np.asarray(edge_index)
    n_real, in_c = x.shape
    d = w_proj.shape[1]
    h, c = att.shape[1], att.shape[2]
    nlayer = w_l.shape[0]
    ncls = w_pred.shape[1]

    nl = _ceil_to(_ceil_to(n_real, ncores) // ncores, 128)
    ngrp = nl // 128
    npad = nl * ncores
    ch = npad // NCHUNK
    assert npad % NCHUNK == 0 and ch <= 32768
    nfake = npad - n_real
    assert nfake >= NCHUNK - 1

    # node -> core: contiguous real ranges; cores 1,3,5 donate one slot to a
    # fake node (their last table row becomes that chunk's poison row), core 7
    # takes the rest of the fakes (incl. chunk 3's poison row).
    fakes_per_core = [0, 1, 0, 1, 0, 1, 0, nfake - 3]
    real_counts = [nl - f for f in fakes_per_core]
    assert sum(real_counts) == n_real
    bounds = np.concatenate([[0], np.cumsum(real_counts)])
    node_core = np.empty(n_real, np.int64)
    for k in range(ncores):
        node_core[bounds[k]:bounds[k + 1]] = k

    src = edge_index[0].astype(np.int64)
    dst = edge_index[1].astype(np.int64)
    src_chunk = node_core[src] // 2
    prof = np.zeros((n_real, NCHUNK), np.int32)
    np.add.at(prof, (dst, src_chunk), 1)
    assert prof.sum(axis=1).min() >= 1, "zero-degree real node"

    # pad poison must dominate logits
    att_flat = att.reshape(nlayer, h * c)
    min_att_sum = np.abs(att_flat).reshape(nlayer, h, c).sum(-1).min()
    assert min_att_sum * NEG_SLOPE * BIG > 30, f"pad poison too weak: {min_att_sum}"

    # per-core packing
    nodes_by_lid = np.empty((ncores, nl), np.int64)
    gmax_all = np.zeros((ncores, ngrp, NCHUNK), np.int64)
    fake_ids = iter(range(n_real, n_real + nfake))
    for k in range(ncores):
        ids_k = np.arange(bounds[k], bounds[k + 1])
        assign, gmax = _greedy_pack(prof[ids_k], ngrp)
        # sort groups by width-sum desc (cross-core alignment)
        gorder = np.argsort(-gmax.sum(axis=1), kind="stable")
        rank_of = np.empty(ngrp, np.int64)
        rank_of[gorder] = np.arange(ngrp)
        groups = [[] for _ in range(ngrp)]
        for i, g in enumerate(assign):
            groups[rank_of[g]].append(ids_k[i])
        gmax = gmax[gorder]
        nf = fakes_per_core[k]
        if nf > 0 and len(groups[-1]) == 128:
            # make room in the last group: move one member to a deficit group
            j = next(jj for jj in range(ngrp) if len(groups[jj]) < 128)
            mv = groups[-1].pop()
            groups[j].append(mv)
            gmax[j] = np.maximum(gmax[j], prof[mv])
        # fill deficits with fakes, last group last (poison row = last slot)
        for g in range(ngrp - 1, -1, -1):
            while len(groups[g]) < 128:
                groups[g].append(next(fake_ids))
        nodes_by_lid[k] = np.concatenate([np.asarray(g, np.int64) for g in groups])
        gmax_all[k] = gmax
        if nf > 0:
            assert nodes_by_lid[k, nl - 1] >= n_real, "poison row must be fake"

    wgc = gmax_all.max(axis=0)  # [ngrp, 4] cross-core widths

    # runs: contiguous group ranks sharing quantized widths
    runs = []
    g0 = 0
    colbase = 0
    while g0 < ngrp:
        R = 1
        rw = wgc[g0].copy()
        while g0 + R < ngrp:
            rw2 = np.maximum(rw, wgc[g0 + R])
            if (R + 1) * rw2.sum() > COLS_MAX:
                break
            rw = rw2
            R += 1
        runs.append((g0, R, tuple(int(w) for w in rw), colbase))
        colbase += R * int(rw.sum())
        g0 += R
    sw = colbase
    wcmax = max(max(r[2]) for r in runs)

    # table positions
    tab_pos = np.empty(npad, np.int64)
    for k in range(ncores):
        tab_pos[nodes_by_lid[k]] = k * nl + np.arange(nl)

    # per-edge slot assignment
    run_of_g = np.empty(ngrp, np.int64)
    for ri, (rg0, R, rw, cb) in enumerate(runs):
        run_of_g[rg0:rg0 + R] = ri
    run_arr = np.array([(rg0, R, cb) for (rg0, R, rw, cb) in runs], np.int64)
    rw_arr = np.array([r[2] for r in runs], np.int64)              # [nrun, 4]
    rcum = np.concatenate([np.zeros((len(runs), 1), np.int64),
                           np.cumsum(rw_arr, axis=1)], axis=1)     # [nrun, 5]

    k_dst = node_core[dst]
    lid = tab_pos[dst] - k_dst * nl
    p_e = lid % 128
    g_e = lid // 128
    ri_e = run_of_g[g_e]
    # j = rank of edge within (dst, chunk)
    eo = np.lexsort((src_chunk, dst))
    ds, cs = dst[eo], src_chunk[eo]
    key = ds * NCHUNK + cs
    _, starts, counts = np.unique(key, return_index=True, return_counts=True)
    j_s = np.arange(len(eo)) - np.repeat(starts, counts)
    j_e = np.empty(len(eo), np.int64)
    j_e[eo] = j_s
    assert (j_e < rw_arr[ri_e, src_chunk]).all()
    col = (run_arr[ri_e, 2]                                  # run colbase
           + (g_e - run_arr[ri_e, 0]) * rw_arr[ri_e, src_chunk]
           + rcum[ri_e, src_chunk] * run_arr[ri_e, 1]        # region start
           + j_e)
    val = (tab_pos[src] - src_chunk * ch).astype(np.int16)
    assert (tab_pos[src] // ch == src_chunk).all()

    # int16 index arrays: slot (p, col) -> idx16[p%16, 8*col + p//16]
    idx16 = np.full((ncores, 16, 8 * sw), ch - 1, np.int16)  # default: poison
    idx16[k_dst, p_e % 16, 8 * col + p_e // 16] = val

    # x, transposed + padded, with ones row for the bias trick
    inp = _ceil_to(in_c + 1, 128)
    per_core = []
    for k in range(ncores):
        xt = np.zeros((inp, nl), np.float32)
        nodes = nodes_by_lid[k]
        real = nodes < n_real
        xt[:in_c, real] = x[nodes[real]].T
        xt[in_c, :] = 1.0
        per_core.append({"x_t": xt,
                         "idx_all": np.ascontiguousarray(np.tile(idx16[k], (8, 1)))})

    wp_pad = np.zeros((inp, d), np.float32)
    wp_pad[:in_c] = w_proj
    wp_pad[in_c] = b_proj
    shared = {"w_proj": wp_pad,
              "w_pred": np.vstack([w_pred, b_pred[None, :]]).astype(np.float32)}
    for l in range(nlayer):
        shared[f"wl{l}"] = np.vstack([w_l[l], b_l[l][None, :]]).astype(np.float32)
        shared[f"wr{l}"] = np.vstack([w_r[l], b_r[l][None, :]]).astype(np.float32)
        shared[f"att{l}"] = np.broadcast_to(
            att_flat[l][None, None, :], (128, wcmax, h * c)
        ).reshape(128, wcmax * h * c).astype(np.float32)
        shared[f"cb{l}"] = np.broadcast_to(conv_bias[l][None, :], (128, h * c)).astype(np.float32)
        shared[f"padv{l}"] = np.broadcast_to(
            (-np.sign(att_flat[l]) * BIG).astype(np.float32)[None, :], (NCHUNK, h * c)
        ).copy()

    cfg = Cfg(ncores=ncores, n_real=n_real, nl=nl, ngrp=ngrp, inp=inp, in_c=in_c,
              d=d, h=h, c=c, ncls=ncls, nlayer=nlayer, npad=npad, ch=ch,
              runs=tuple(runs), sw=sw, wcmax=wcmax)
    meta = {"nodes_by_lid": nodes_by_lid}
    return cfg, per_core, shared, meta


def build_program(cfg: Cfg):
    import os
    ablate = set(os.environ.get("KERN_ABLATE", "").split(","))
    import concourse.bass as bass
    import concourse.bacc as bacc
    import concourse.mybir as mybir
    import concourse.tile as tile
    from concourse.masks import make_identity
    from concourse.tile import add_dep_helper

    f32 = mybir.dt.float32
    i16 = mybir.dt.int16
    P = 128
    D, H, C = cfg.d, cfg.h, cfg.c
    NGRP, NL, CH = cfg.ngrp, cfg.nl, cfg.ch
    HS = D + 1  # h chunk stride (extra ones column for the bias-row trick)
    EW = 2 * D  # gather row width in f32 (256B)

    nc = bacc.Bacc(trn_type="TRN2", num_devices=cfg.ncores)

    x_t = nc.dram_tensor("x_t", [cfg.inp, NL], f32, kind="ExternalInput")
    idx_in = nc.dram_tensor("idx_all", [P, 8 * cfg.sw], i16, kind="ExternalInput")
    wp_in = nc.dram_tensor("w_proj", [cfg.inp, D], f32, kind="ExternalInput")
    wpred_in = nc.dram_tensor("w_pred", [D + 1, cfg.ncls], f32, kind="ExternalInput")
    wl_in = [nc.dram_tensor(f"wl{l}", [D + 1, D], f32, kind="ExternalInput") for l in range(cfg.nlayer)]
    wr_in = [nc.dram_tensor(f"wr{l}", [D + 1, D], f32, kind="ExternalInput") for l in range(cfg.nlayer)]
    att_in = [nc.dram_tensor(f"att{l}", [P, cfg.wcmax * D], f32, kind="ExternalInput") for l in range(cfg.nlayer)]
    cb_in = [nc.dram_tensor(f"cb{l}", [P, D], f32, kind="ExternalInput") for l in range(cfg.nlayer)]
    padv_in = [nc.dram_tensor(f"padv{l}", [NCHUNK, D], f32, kind="ExternalInput") for l in range(cfg.nlayer)]
    out_dram = nc.dram_tensor("out", [P, NGRP * cfg.ncls], f32, kind="ExternalOutput")
    dbg_h = (nc.dram_tensor("dbg_h", [P, NGRP * (D + 1)], f32, kind="ExternalOutput")
             if cfg.debug else None)

    xl_own = nc.dram_tensor("xl_own", [NL, D], f32)
    if cfg.ncores > 4:  # shared-output collectives need >4 cores
        xl_allg = nc.dram_tensor("xl_allg", [cfg.npad, D], f32, addr_space="Shared")
    else:
        xl_allg = nc.dram_tensor("xl_allg", [cfg.npad, D], f32)
    # gather table: 256B rows, one spare row so the last row's 256B read is safe
    xl_tab = nc.dram_tensor("xl_tab", [cfg.npad + 1, EW], f32)

    with tile.TileContext(nc) as tc:
        with (
            tc.tile_pool(name="const", bufs=1) as cp,
            tc.tile_pool(name="pers", bufs=1) as pp,
            tc.tile_pool(name="work", bufs=3) as wp,
            tc.tile_pool(name="gath", bufs=2) as gp,
            tc.tile_pool(name="edge", bufs=1) as ep,
            tc.tile_pool(name="small", bufs=2) as sp,
            tc.tile_pool(name="ps_mm", bufs=2, space="PSUM") as pmm,
            tc.tile_pool(name="ps_tr", bufs=2, space="PSUM") as ptr,
        ):
            # ---- constants -> SBUF ----
            ident = cp.tile([P, P], f32)
            make_identity(nc, ident[:])
            nj = cfg.inp // P
            wp_sb = cp.tile([P, nj * D], f32)
            nc.sync.dma_start(out=wp_sb[:].rearrange("p (j d) -> p j d", d=D),
                              in_=wp_in[:].rearrange("(j p) d -> p j d", p=P))
            wl_sb = [cp.tile([D + 1, D], f32, name=f"wl_sb{l}") for l in range(cfg.nlayer)]
            wr_sb = [cp.tile([D + 1, D], f32, name=f"wr_sb{l}") for l in range(cfg.nlayer)]
            cb_sb = [cp.tile([P, D], f32, name=f"cb_sb{l}") for l in range(cfg.nlayer)]
            for l in range(cfg.nlayer):
                nc.sync.dma_start(out=wl_sb[l][:], in_=wl_in[l][:])
                nc.sync.dma_start(out=wr_sb[l][:], in_=wr_in[l][:])
                nc.sync.dma_start(out=cb_sb[l][:], in_=cb_in[l][:])
            wpred_sb = cp.tile([D + 1, cfg.ncls], f32)
            nc.sync.dma_start(out=wpred_sb[:], in_=wpred_in[:])
            att_sb = pp.tile([P, cfg.wcmax * D], f32)   # reloaded per layer
            ones_sb = cp.tile([P, 1], f32)
            nc.gpsimd.memset(ones_sb[:], 1.0)

            h_a = pp.tile([P, NGRP * HS], f32, name="h_a")
            h_b = pp.tile([P, NGRP * HS], f32, name="h_b")
            xr_own = pp.tile([P, NGRP * D], f32)

            def h_view(t):  # [P, NGRP, D] data columns
                return t[:].rearrange("p (g s) -> p g s", s=HS)[:, :, :D]

            def ones_col(t):
                return t[:].rearrange("p (g s) -> p g s", s=HS)[:, :, D:HS]

            # ---- P1: h0 = x @ w_proj + b_proj ----
            col_tiles = []
            c0 = 0
            while c0 < NL:
                tw = min(512, NL - c0)
                col_tiles.append((c0, tw))
                c0 += tw
            for (c0, tw) in col_tiles:
                h_acc = pmm.tile([D, 512], f32, tag="h_acc")
                for jj in range(nj):
                    xtile = wp.tile([P, 512], f32, tag="xtile")
                    nc.sync.dma_start(out=xtile[:, :tw], in_=x_t[jj * P:(jj + 1) * P, c0:c0 + tw])
                    nc.tensor.matmul(out=h_acc[:, :tw], lhsT=wp_sb[:, jj * D:(jj + 1) * D],
                                     rhs=xtile[:, :tw], start=(jj == 0), stop=(jj == nj - 1))
                hT_stage = wp.tile([D, 512], f32, tag="hT_stage")
                nc.scalar.copy(out=hT_stage[:, :tw], in_=h_acc[:, :tw])
                for t2 in range(tw // P):
                    chk = (c0 + t2 * P) // P
                    htr = ptr.tile([P, D], f32, tag="htr", bufs=1)
                    nc.tensor.transpose(out=htr[:], in_=hT_stage[:, t2 * P:(t2 + 1) * P],
                                        identity=ident[:D, :D])
                    nc.vector.tensor_copy(out=h_view(h_a)[:, chk, :], in_=htr[:])
            nc.vector.tensor_copy(out=ones_col(h_a)[:, :, 0], in_=ones_sb[:].to_broadcast([P, NGRP]))

            h_cur, h_nxt = h_a, h_b

            # ---- P2: layers ----
            for l in range(cfg.nlayer):
                nc.sync.dma_start(out=att_sb[:], in_=att_in[l][:])
                # (a) xl/xr for owned nodes; xl -> DRAM (+allgather), xr -> SBUF
                xl_dmas = []
                nbatch = math.ceil(NGRP / 16)
                for b in range(nbatch):
                    chunks = range(b * 16, min((b + 1) * 16, NGRP))
                    hT_chs = {}
                    for chk in chunks:
                        tr = ptr.tile([HS, P], f32, tag="tr")
                        nc.tensor.transpose(
                            out=tr[:], in_=h_cur[:].rearrange("p (g s) -> p g s", s=HS)[:, chk, :],
                            identity=ident[:])
                        hT_ch = wp.tile([HS, P], f32, tag="hT_ch")
                        nc.scalar.copy(out=hT_ch[:], in_=tr[:])
                        hT_chs[chk] = hT_ch
                    for (dst_sb, w_t, to_dram) in ((None, wl_sb[l], True), (xr_own, wr_sb[l], False)):
                        big = pmm.tile([P, 512], f32, tag="big")
                        for i, chk in enumerate(chunks):
                            nc.tensor.matmul(out=big[:, i * D:(i + 1) * D], lhsT=hT_chs[chk][:],
                                             rhs=w_t[:], start=True, stop=True)
                        ncols = len(chunks) * D
                        if to_dram:
                            stage = wp.tile([P, 512], f32, tag="xl_stage")
                            nc.scalar.copy(out=stage[:, :ncols], in_=big[:, :ncols])
                            dma = nc.sync.dma_start(
                                out=xl_own[:].rearrange("(a p) d -> p a d", p=P)[
                                    :, b * 16:b * 16 + len(chunks), :],
                                in_=stage[:, :ncols].rearrange("p (a d) -> p a d", d=D))
                            xl_dmas.append(dma)
                        else:
                            nc.scalar.copy(out=dst_sb[:, b * 16 * D: b * 16 * D + ncols],
                                           in_=big[:, :ncols])
                # (b) allgather xl; expand into 256B-stride table; poison rows
                if "allg" in ablate:
                    pass
                elif cfg.ncores > 1:
                    cc = nc.gpsimd.collective_compute(
                        "AllGather", mybir.AluOpType.bypass,
                        replica_groups=[list(range(cfg.ncores))],
                        ins=[xl_own[:]], outs=[xl_allg[:]])
                    for dma in xl_dmas:
                        add_dep_helper(cc.ins, dma.ins)
                    cpls = []
                    for ci in range(NCHUNK):
                        if "copy" in ablate:
                            break
                        cpl = nc.sync.dma_start(
                            out=xl_tab[ci * CH:(ci + 1) * CH, :D],
                            in_=xl_allg[ci * CH:(ci + 1) * CH, :])
                        add_dep_helper(cpl.ins, cc.ins)
                        cpls.append(cpl)
                else:
                    cpls = []
                    for ci in range(NCHUNK):
                        lo, hi = ci * CH, min((ci + 1) * CH, NL)
                        if lo >= hi:
                            continue
                        cpl = nc.sync.dma_start(out=xl_tab[lo:hi, :D],
                                                in_=xl_own[lo:hi, :])
                        for dma in xl_dmas:
                            add_dep_helper(cpl.ins, dma.ins)
                        cpls.append(cpl)
                if "allg" in ablate or "copy" in ablate:
                    pv = None
                else:
                    pv = nc.sync.dma_start(
                        out=xl_tab[0:cfg.npad, :].rearrange("(a r) e -> a r e", r=CH)[:, CH - 1, :D],
                        in_=padv_in[l][:])
                    for cpl in cpls:
                        add_dep_helper(pv.ins, cpl.ins)

                # (c) edge phase per run
                for (g0, R, rw, colbase) in (() if "edge" in ablate else cfg.runs):
                    cols = R * sum(rw)
                    rcum = [0]
                    for w in rw:
                        rcum.append(rcum[-1] + w)
                    idx_sb = gp.tile([P, 8 * COLS_MAX], i16, tag="idx")
                    nc.sync.dma_start(out=idx_sb[:, :8 * cols],
                                      in_=idx_in[:, 8 * colbase: 8 * (colbase + cols)])
                    xg = gp.tile([P, COLS_MAX * EW], f32, tag="xg")
                    xgq = xg[:].rearrange("p (q e) -> p q e", e=EW)
                    for cidx in range(NCHUNK):
                        W = rw[cidx]
                        if W == 0:
                            continue
                        rs = R * rcum[cidx]
                        ncol = R * W
                        # dma_gather is only reliable up to 1024 idxs/call
                        # (HW-probed); split into <=8-column calls.
                        for off in range(0, ncol, 8):
                            if "gather" in ablate:
                                break
                            nn = min(8, ncol - off)
                            gth = nc.gpsimd.dma_gather(
                                out_ap=xgq[:, rs + off:rs + off + nn, :],
                                in_ap=xl_tab[cidx * CH:(cidx + 1) * CH, :],
                                idxs_ap=idx_sb[:, 8 * (rs + off): 8 * (rs + off + nn)],
                                num_idxs=128 * nn,
                                num_idxs_reg=128 * nn,
                                elem_size=EW,
                            )
                            if pv is not None:
                                add_dep_helper(gth.ins, pv.ins)
                    if "dve" in ablate:
                        continue
                    ex = ep.tile([P, COLS_MAX * H], f32, tag="ex")
                    exq = ex[:].rearrange("p (q h) -> p q h", h=H)
                    # per chunk-region: s = xl + xr (into upper half), leaky,
                    # *att, logit reduce
                    for cidx in range(NCHUNK):
                        W = rw[cidx]
                        if W == 0:
                            continue
                        rs = R * rcum[cidx]
                        ncol = R * W
                        reg = xgq[:, rs:rs + ncol, :]
                        reg_r = reg.rearrange("p (r w) e -> p r w e", w=W)
                        nc.vector.tensor_tensor(
                            out=reg_r[:, :, :, D:EW],
                            in0=reg_r[:, :, :, 0:D],
                            in1=xr_own[:].rearrange("p (g d) -> p g d", d=D)[
                                :, g0:g0 + R, :].unsqueeze(2).to_broadcast([P, R, W, D]),
                            op=mybir.AluOpType.add)
                        s_flat = reg[:, :, D:EW]
                        nc.vector.scalar_tensor_tensor(
                            out=s_flat, in0=s_flat, scalar=NEG_SLOPE, in1=s_flat,
                            op0=mybir.AluOpType.mult, op1=mybir.AluOpType.max)
                        nc.vector.tensor_tensor(
                            out=reg_r[:, :, :, D:EW], in0=reg_r[:, :, :, D:EW],
                            in1=att_sb[:].rearrange("p (w d) -> p w d", d=D)[
                                :, :W, :].unsqueeze(1).to_broadcast([P, R, W, D]),
                            op=mybir.AluOpType.mult)
                        nc.vector.tensor_reduce(
                            out=exq[:, rs:rs + ncol, :],
                            in_=reg[:, :, D:EW].rearrange("p q (hh cc) -> p q hh cc", cc=C),
                            axis=mybir.AxisListType.X, op=mybir.AluOpType.add)
                        nc.scalar.activation(
                            out=ex[:, rs * H:(rs + ncol) * H],
                            in_=ex[:, rs * H:(rs + ncol) * H],
                            func=mybir.ActivationFunctionType.Exp)
                    # denominators: per chunk-region reduce over W, then sum
                    den4 = sp.tile([P, NCHUNK * R * H], f32, tag="den4")
                    nreg = 0
                    for cidx in range(NCHUNK):
                        W = rw[cidx]
                        if W == 0:
                            continue
                        rs = R * rcum[cidx]
                        nc.vector.tensor_reduce(
                            out=den4[:, nreg * R * H:(nreg + 1) * R * H],
                            in_=exq[:, rs:rs + R * W, :].rearrange(
                                "p (r w) h -> p r h w", w=W),
                            axis=mybir.AxisListType.X, op=mybir.AluOpType.add)
                        nreg += 1
                    den = sp.tile([P, R * H], f32, tag="den")
                    nc.vector.tensor_reduce(
                        out=den[:],
                        in_=den4[:, :nreg * R * H].rearrange(
                            "p (n q) -> p q n", n=nreg),
                        axis=mybir.AxisListType.X, op=mybir.AluOpType.add)
                    nc.vector.tensor_scalar_max(den[:], den[:], 1e-16)
                    rec = sp.tile([P, R * H], f32, tag="rec")
                    nc.vector.reciprocal(out=rec[:], in_=den[:])
                    # weighted aggregation: tmp = xl * ex (in place over xl),
                    # numer = sum_w tmp
                    num4 = sp.tile([P, NCHUNK * R * D], f32, tag="num4")
                    nreg = 0
                    for cidx in range(NCHUNK):
                        W = rw[cidx]
                        if W == 0:
                            continue
                        rs = R * rcum[cidx]
                        ncol = R * W
                        reg = xgq[:, rs:rs + ncol, :]
                        nc.vector.tensor_tensor(
                            out=reg[:, :, 0:D].rearrange("p q (hh cc) -> p q hh cc", cc=C),
                            in0=reg[:, :, 0:D].rearrange("p q (hh cc) -> p q hh cc", cc=C),
                            in1=exq[:, rs:rs + ncol, :].unsqueeze(3).to_broadcast(
                                [P, ncol, H, C]),
                            op=mybir.AluOpType.mult)
                        nc.vector.tensor_reduce(
                            out=num4[:, nreg * R * D:(nreg + 1) * R * D],
                            in_=reg[:, :, 0:D].rearrange("p (r w) d -> p r d w", w=W),
                            axis=mybir.AxisListType.X, op=mybir.AluOpType.add)
                        nreg += 1
                    numer = sp.tile([P, R * D], f32, tag="numer")
                    nc.vector.tensor_reduce(
                        out=numer[:],
                        in_=num4[:, :nreg * R * D].rearrange(
                            "p (n q) -> p q n", n=nreg),
                        axis=mybir.AxisListType.X, op=mybir.AluOpType.add)
                    o1 = sp.tile([P, R * D], f32, tag="o1")
                    nc.vector.tensor_tensor(
                        out=o1[:].rearrange("p (r hh cc) -> p r hh cc", r=R, cc=C),
                        in0=numer[:].rearrange("p (r hh cc) -> p r hh cc", r=R, cc=C),
                        in1=rec[:].rearrange("p (r hh) -> p r hh", r=R).unsqueeze(
                            3).to_broadcast([P, R, H, C]),
                        op=mybir.AluOpType.mult)
                    o3 = sp.tile([P, R * D], f32, tag="o3")
                    nc.vector.tensor_tensor(
                        out=o3[:].rearrange("p (r d) -> p r d", d=D),
                        in0=o1[:].rearrange("p (r d) -> p r d", d=D),
                        in1=h_view(h_cur)[:, g0:g0 + R, :],
                        op=mybir.AluOpType.add)
                    nc.vector.tensor_tensor(
                        out=h_view(h_nxt)[:, g0:g0 + R, :],
                        in0=o3[:].rearrange("p (r d) -> p r d", d=D),
                        in1=cb_sb[l][:].unsqueeze(1).to_broadcast([P, R, D]),
                        op=mybir.AluOpType.add)
                nc.vector.tensor_copy(out=ones_col(h_nxt)[:, :, 0],
                                      in_=ones_sb[:].to_broadcast([P, NGRP]))
                h_cur, h_nxt = h_nxt, h_cur

            if dbg_h is not None:
                nc.sync.dma_start(out=dbg_h[:], in_=h_cur[:])

            # ---- P3: out = h2 @ w_pred + b_pred ----
            nbatch = math.ceil(NGRP / 16)
            for b in range(nbatch):
                chunks = range(b * 16, min((b + 1) * 16, NGRP))
                fin = pmm.tile([P, 16 * cfg.ncls], f32, tag="fin", bufs=1)
                for i, chk in enumerate(chunks):
                    tr = ptr.tile([HS, P], f32, tag="tr")
                    nc.tensor.transpose(
                        out=tr[:], in_=h_cur[:].rearrange("p (g s) -> p g s", s=HS)[:, chk, :],
                        identity=ident[:])
                    hT_ch = wp.tile([HS, P], f32, tag="hT_ch")
                    nc.scalar.copy(out=hT_ch[:], in_=tr[:])
                    nc.tensor.matmul(out=fin[:, i * cfg.ncls:(i + 1) * cfg.ncls], lhsT=hT_ch[:],
                                     rhs=wpred_sb[:], start=True, stop=True)
                ncols = len(chunks) * cfg.ncls
                ostage = wp.tile([P, 16 * cfg.ncls], f32, tag="ostage")
                nc.scalar.copy(out=ostage[:, :ncols], in_=fin[:, :ncols])
                nc.sync.dma_start(out=out_dram[:, b * 16 * cfg.ncls: b * 16 * cfg.ncls + ncols],
                                  in_=ostage[:, :ncols])
    nc.finalize()
    return nc


def assemble_output(cfg: Cfg, meta, core_outs):
    """core_outs: list of [128, NGRP*ncls] arrays -> full [n_real, ncls]."""
    full = np.zeros((cfg.npad, cfg.ncls), np.float32)
    for k in range(cfg.ncores):
        o = core_outs[k].reshape(128, cfg.ngrp, cfg.ncls).transpose(1, 0, 2).reshape(cfg.nl, cfg.ncls)
        full[meta["nodes_by_lid"][k]] = o
    return full[:cfg.n_real]


_LAST = {}


def bench(inputs, iters=20) -> dict:
    """Correctness + repeat-execution timing via a hand-rolled PJRT runner
    (no NTFF hook in this environment). Returns output + per-iter seconds."""
    import time

    import jax
    from jax.sharding import Mesh, NamedSharding, PartitionSpec
    from jax.experimental.shard_map import shard_map
    import concourse.mybir as mybir
    from concourse import bass2jax
    from concourse.bass2jax import _bass_exec_p, install_neuronx_cc_hook, partition_id_tensor

    inputs = {k: np.asarray(v) for k, v in inputs.items()}
    cfg, per_core, shared, meta = host_prep(ncores=NCORES, **inputs)
    nc = build_program(cfg)
    in_maps = [{**shared, **pc} for pc in per_core]

    install_neuronx_cc_hook()
    partition_name = nc.partition_id_tensor.name if nc.partition_id_tensor else None
    in_names, out_names, out_avals, zero_outs = [], [], [], []
    for alloc in nc.m.functions[0].allocations:
        if not isinstance(alloc, mybir.MemoryLocationSet):
            continue
        name = alloc.memorylocations[0].name
        if alloc.kind == "ExternalInput":
            if name != partition_name:
                in_names.append(name)
        elif alloc.kind == "ExternalOutput":
            shape = tuple(alloc.tensor_shape)
            dtype = mybir.dt.np(alloc.dtype)
            out_names.append(name)
            out_avals.append(jax.core.ShapedArray(shape, dtype))
            zero_outs.append(np.zeros(shape, dtype))
    n_params = len(in_names)
    n_outs = len(out_avals)
    all_in_names = list(in_names) + list(out_names)
    if partition_name is not None:
        all_in_names.append(partition_name)
    donate = tuple(range(n_params, n_params + n_outs))

    def _body(*args):
        operands = list(args)
        if partition_name is not None:
            operands.append(partition_id_tensor())
        return tuple(_bass_exec_p.bind(
            *operands, out_avals=tuple(out_avals), in_names=tuple(all_in_names),
            out_names=tuple(out_names), lowering_input_output_aliases=(),
            sim_require_finite=True, sim_require_nnan=True, nc=nc))

    devices = jax.devices()[:NCORES]
    mesh = Mesh(np.asarray(devices), ("core",))
    in_specs = (PartitionSpec("core"),) * (n_params + n_outs)
    out_specs = (PartitionSpec("core"),) * n_outs
    sharded = jax.jit(shard_map(_body, mesh=mesh, in_specs=in_specs,
                                out_specs=out_specs, check_rep=False),
                      donate_argnums=donate, keep_unused=True)
    concat_in = [np.concatenate([np.asarray(in_maps[c][n]) for c in range(NCORES)], axis=0)
                 for n in in_names]
    t0 = time.time()
    sh = NamedSharding(mesh, PartitionSpec("core"))
    args_dev = [jax.device_put(a, sh) for a in concat_in]
    jax.block_until_ready(args_dev)
    t_put = time.time() - t0

    def zeros_dev():
        return [jax.device_put(np.zeros((NCORES * z.shape[0], *z.shape[1:]), z.dtype), sh)
                for z in zero_outs]

    t0 = time.time()
    outs = sharded(*args_dev, *zeros_dev())
    jax.block_until_ready(outs)
    t_first = time.time() - t0
    result = [
        {name: np.asarray(outs[i]).reshape(NCORES, *out_avals[i].shape)[c]
         for i, name in enumerate(out_names)} for c in range(NCORES)]

    # warm + timed loop (zeros pre-staged on device; donated per call)
    zsets = [zeros_dev() for _ in range(iters + 2)]
    jax.block_until_ready(zsets)
    r = sharded(*args_dev, *zsets[0])
    jax.block_until_ready(r)
    r = sharded(*args_dev, *zsets[1])
    jax.block_until_ready(r)
    t0 = time.time()
    rs = [sharded(*args_dev, *zsets[2 + i]) for i in range(iters)]
    jax.block_until_ready(rs)
    per_iter = (time.time() - t0) / iters

    out_full = assemble_output(cfg, meta, [result[k]["out"] for k in range(NCORES)])
    return {"out": out_full, "per_iter_s": per_iter, "first_s": t_first,
            "put_s": t_put, "cfg": cfg}


def kernel(**inputs) -> np.ndarray:
    import time

    from concourse.bass_utils import run_bass_kernel_spmd

    inputs = {k: np.asarray(v) for k, v in inputs.items()}
    cfg, per_core, shared, meta = host_prep(ncores=NCORES, **inputs)
    nc = build_program(cfg)
    in_maps = [{**shared, **pc} for pc in per_core]
    res = None
    for attempt in range(2):
        try:
            res = run_bass_kernel_spmd(nc, in_maps, core_ids=list(range(NCORES)))
            break
        except Exception:
            # transient device wedge (NRT_EXEC_UNIT_UNRECOVERABLE) recovers on
            # a fresh attempt once the runtime re-initializes
            if attempt == 1:
                raise
            time.sleep(20)
    assert res is not None
    _LAST["res"] = res
    _LAST["cfg"] = cfg
    outs = [res.results[k]["out"] for k in range(NCORES)]
    return assemble_output(cfg, meta, outs)


# revision 16
# speedup vs baseline: 4.0796x; 2.0493x over previous
"""GATv2 (2-layer, 8-head) message-passing kernel for Trainium2, 8 NeuronCores.

v2: batched SWDGE dma_gather replaces per-column indirect DMA.

Strategy (sharding_hint: partition edges by destination, nodes by range):
- Host: nodes split across 8 cores; per core, nodes are packed into 98 groups
  of 128 destination nodes via greedy bin-packing that minimizes the summed
  per-chunk max in-degree (the gather is split into 4 source-range chunks
  because dma_gather indices are int16, table has 100352 rows). Groups are
  coalesced into "runs" sharing quantized per-chunk widths so one dma_gather
  per (run, chunk) fetches all payload rows for several groups at once.
- Device, per core: project x -> h (PE); per layer compute xl/xr for owned
  nodes (PE), AllGather the xl table, expand it into a 256B-stride gather
  table, write 4 per-chunk poison rows (pad slots gather them; their logits
  land <= -6e3 so exp()==0), then per run: 4 dma_gathers + a short DVE chain
  (s=xl+xr, leaky-relu, *att, reduce-C, exp on ACT, softmax-normalize,
  weighted aggregation with free-dim reduces).
"""

import math
from dataclasses import dataclass

import numpy as np

# ---- problem constants (hardcoded; harness calls kernel(**inputs) directly) --
N = 100000
E = 3200000
IN_C = 1433
DIM = 32
HEADS = 8
OUT_C = 4
NUM_LAYER = 2
NUM_CLASS = 7
NEG_SLOPE = 0.2
NCORES = 8
BIG = 3.0e4
NCHUNK = 4
COLS_MAX = 208  # max gather-payload columns per run batch (SBUF budget)


@dataclass
class Cfg:
    ncores: int
    n_real: int
    nl: int           # owned (padded) nodes per core, = ngrp*128
    ngrp: int
    inp: int          # padded input feature dim (multiple of 128, >= IN_C+1)
    in_c: int
    d: int            # DIM
    h: int            # HEADS
    c: int            # OUT_C
    ncls: int
    nlayer: int
    npad: int
    ch: int           # chunk size (npad // 4)
    runs: tuple       # per run: (g0, R, (w0,w1,w2,w3), colbase)
    sw: int           # total payload columns (sum over runs of R*sum(w))
    wcmax: int        # max chunk width
    debug: bool = False


def _ceil_to(x, m):
    return (x + m - 1) // m * m


def _greedy_pack(prof_k, ngrp):
    """Pack len(prof_k) nodes (rows of [4] chunk profiles) into ngrp groups of
    <=128 minimizing sum_g sum_c max. Returns (assign, gmax)."""
    n = prof_k.shape[0]
    order = np.argsort(-prof_k.max(axis=1), kind="stable")
    gmax = np.zeros((ngrp, NCHUNK), np.int64)
    gcnt = np.zeros(ngrp, np.int64)
    assign = np.empty(n, np.int64)
    for i in order:
        pi = prof_k[i]
        inc = np.maximum(gmax, pi[None, :]).sum(axis=1) - gmax.sum(axis=1)
        score = inc * 1000 - gcnt
        score[gcnt >= 128] = 1 << 50
        g = int(np.argmin(score))
        assign[i] = g
        gmax[g] = np.maximum(gmax[g], pi)
        gcnt[g] += 1
    return assign, gmax


def host_prep(x, edge_index, w_proj, b_proj, w_l, b_l, w_r, b_r, att, conv_bias,
              w_pred, b_pred, ncores):
    """Numpy preprocessing: node->core/group assignment, chunked gather grid,
    int16 index arrays, padded/transposed inputs."""
    x = np.asarray(x)
    edge_index = np.asarray(edge_index)
    n_real, in_c = x.shape
    d = w_proj.shape[1]
    h, c = att.shape[1], att.shape[2]
    nlayer = w_l.shape[0]
    ncls = w_pred.shape[1]

    nl = _ceil_to(_ceil_to(n_real, ncores) // ncores, 128)
    ngrp = nl // 128
    npad = nl * ncores
    ch = npad // NCHUNK
    assert npad % NCHUNK == 0 and ch <= 32768
    nfake = npad - n_real
    assert nfake >= NCHUNK - 1

    # node -> core: contiguous real ranges; cores 1,3,5 donate one slot to a
    # fake node (their last table row becomes that chunk's poison row), core 7
    # takes the rest of the fakes (incl. chunk 3's poison row).
    fakes_per_core = [0, 1, 0, 1, 0, 1, 0, nfake - 3]
    real_counts = [nl - f for f in fakes_per_core]
    assert sum(real_counts) == n_real
    bounds = np.concatenate([[0], np.cumsum(real_counts)])
    node_core = np.empty(n_real, np.int64)
    for k in range(ncores):
        node_core[bounds[k]:bounds[k + 1]] = k

    src = edge_index[0].astype(np.int64)
    dst = edge_index[1].astype(np.int64)
    src_chunk = node_core[src] // 2
    prof = np.zeros((n_real, NCHUNK), np.int32)
    np.add.at(prof, (dst, src_chunk), 1)
    assert prof.sum(axis=1).min() >= 1, "zero-degree real node"

    # pad poison must dominate logits
    att_flat = att.reshape(nlayer, h * c)
    min_att_sum = np.abs(att_flat).reshape(nlayer, h, c).sum(-1).min()
    assert min_att_sum * NEG_SLOPE * BIG > 30, f"pad poison too weak: {min_att_sum}"

    # per-core packing
    nodes_by_lid = np.empty((ncores, nl), np.int64)
    gmax_all = np.zeros((ncores, ngrp, NCHUNK), np.int64)
    fake_ids = iter(range(n_real, n_real + nfake))
    for k in range(ncores):
        ids_k = np.arange(bounds[k], bounds[k + 1])
        assign, gmax = _greedy_pack(prof[ids_k], ngrp)
        # sort groups by width-sum desc (cross-core alignment)
        gorder = np.argsort(-gmax.sum(axis=1), kind="stable")
        rank_of = np.empty(ngrp, np.int64)
        rank_of[gorder] = np.arange(ngrp)
        groups = [[] for _ in range(ngrp)]
        for i, g in enumerate(assign):
            groups[rank_of[g]].append(ids_k[i])
        gmax = gmax[gorder]
        nf = fakes_per_core[k]
        if nf > 0 and len(groups[-1]) == 128:
            # make room in the last group: move one member to a deficit group
            j = next(jj for jj in range(ngrp) if len(groups[jj]) < 128)
            mv = groups[-1].pop()
            groups[j].append(mv)
            gmax[j] = np.maximum(gmax[j], prof[mv])
        # fill deficits with fakes, last group last (poison row = last slot)
        for g in range(ngrp - 1, -1, -1):
            while len(groups[g]) < 128:
                groups[g].append(next(fake_ids))
        nodes_by_lid[k] = np.concatenate([np.asarray(g, np.int64) for g in groups])
        gmax_all[k] = gmax
        if nf > 0:
            assert nodes_by_lid[k, nl - 1] >= n_real, "poison row must be fake"

    wgc = gmax_all.max(axis=0)  # [ngrp, 4] cross-core widths

    # runs: contiguous group ranks sharing quantized widths
    runs = []
    g0 = 0
    colbase = 0
    while g0 < ngrp:
        R = 1
        rw = wgc[g0].copy()
        while g0 + R < ngrp:
            rw2 = np.maximum(rw, wgc[g0 + R])
            if (R + 1) * rw2.sum() > COLS_MAX:
                break
            rw = rw2
            R += 1
        runs.append((g0, R, tuple(int(w) for w in rw), colbase))
        colbase += R * int(rw.sum())
        g0 += R
    sw = colbase
    wcmax = max(max(r[2]) for r in runs)

    # table positions
    tab_pos = np.empty(npad, np.int64)
    for k in range(ncores):
        tab_pos[nodes_by_lid[k]] = k * nl + np.arange(nl)

    # per-edge slot assignment
    run_of_g = np.empty(ngrp, np.int64)
    for ri, (rg0, R, rw, cb) in enumerate(runs):
        run_of_g[rg0:rg0 + R] = ri
    run_arr = np.array([(rg0, R, cb) for (rg0, R, rw, cb) in runs], np.int64)
    rw_arr = np.array([r[2] for r in runs], np.int64)              # [nrun, 4]
    rcum = np.concatenate([np.zeros((len(runs), 1), np.int64),
                           np.cumsum(rw_arr, axis=1)], axis=1)     # [nrun, 5]

    k_dst = node_core[dst]
    lid = tab_pos[dst] - k_dst * nl
    p_e = lid % 128
    g_e = lid // 128
    ri_e = run_of_g[g_e]
    # j = rank of edge within (dst, chunk)
    eo = np.lexsort((src_chunk, dst))
    ds, cs = dst[eo], src_chunk[eo]
    key = ds * NCHUNK + cs
    _, starts, counts = np.unique(key, return_index=True, return_counts=True)
    j_s = np.arange(len(eo)) - np.repeat(starts, counts)
    j_e = np.empty(len(eo), np.int64)
    j_e[eo] = j_s
    assert (j_e < rw_arr[ri_e, src_chunk]).all()
    col = (run_arr[ri_e, 2]                                  # run colbase
           + (g_e - run_arr[ri_e, 0]) * rw_arr[ri_e, src_chunk]
           + rcum[ri_e, src_chunk] * run_arr[ri_e, 1]        # region start
           + j_e)
    val = (tab_pos[src] - src_chunk * ch).astype(np.int16)
    assert (tab_pos[src] // ch == src_chunk).all()

    # int16 index arrays: slot (p, col) -> idx16[p%16, 8*col + p//16]
    idx16 = np.full((ncores, 16, 8 * sw), ch - 1, np.int16)  # default: poison
    idx16[k_dst, p_e % 16, 8 * col + p_e // 16] = val

    # x, transposed + padded, with ones row for the bias trick
    inp = _ceil_to(in_c + 1, 128)
    per_core = []
    for k in range(ncores):
        xt = np.zeros((inp, nl), np.float32)
        nodes = nodes_by_lid[k]
        real = nodes < n_real
        xt[:in_c, real] = x[nodes[real]].T
        xt[in_c, :] = 1.0
        per_core.append({"x_t": xt,
                         "idx_all": np.ascontiguousarray(np.tile(idx16[k], (8, 1)))})

    wp_pad = np.zeros((inp, d), np.float32)
    wp_pad[:in_c] = w_proj
    wp_pad[in_c] = b_proj
    shared = {"w_proj": wp_pad,
              "w_pred": np.vstack([w_pred, b_pred[None, :]]).astype(np.float32)}
    for l in range(nlayer):
        shared[f"wl{l}"] = np.vstack([w_l[l], b_l[l][None, :]]).astype(np.float32)
        shared[f"wr{l}"] = np.vstack([w_r[l], b_r[l][None, :]]).astype(np.float32)
        shared[f"att{l}"] = np.broadcast_to(
            att_flat[l][None, None, :], (128, wcmax, h * c)
        ).reshape(128, wcmax * h * c).astype(np.float32)
        shared[f"cb{l}"] = np.broadcast_to(conv_bias[l][None, :], (128, h * c)).astype(np.float32)
        shared[f"padv{l}"] = np.broadcast_to(
            (-np.sign(att_flat[l]) * BIG).astype(np.float32)[None, :], (NCHUNK, h * c)
        ).copy()

    cfg = Cfg(ncores=ncores, n_real=n_real, nl=nl, ngrp=ngrp, inp=inp, in_c=in_c,
              d=d, h=h, c=c, ncls=ncls, nlayer=nlayer, npad=npad, ch=ch,
              runs=tuple(runs), sw=sw, wcmax=wcmax)
    meta = {"nodes_by_lid": nodes_by_lid}
    return cfg, per_core, shared, meta


def build_program(cfg: Cfg):
    import os
    ablate = set(os.environ.get("KERN_ABLATE", "").split(","))
    import concourse.bass as bass
    import concourse.bacc as bacc
    import concourse.mybir as mybir
    import concourse.tile as tile
    from concourse.masks import make_identity
    from concourse.tile import add_dep_helper

    f32 = mybir.dt.float32
    i16 = mybir.dt.int16
    P = 128
    D, H, C = cfg.d, cfg.h, cfg.c
    NGRP, NL, CH = cfg.ngrp, cfg.nl, cfg.ch
    HS = D + 1  # h chunk stride (extra ones column for the bias-row trick)
    EW = 2 * D  # gather row width in f32 (256B)

    nc = bacc.Bacc(trn_type="TRN2", num_devices=cfg.ncores)

    x_t = nc.dram_tensor("x_t", [cfg.inp, NL], f32, kind="ExternalInput")
    idx_in = nc.dram_tensor("idx_all", [P, 8 * cfg.sw], i16, kind="ExternalInput")
    wp_in = nc.dram_tensor("w_proj", [cfg.inp, D], f32, kind="ExternalInput")
    wpred_in = nc.dram_tensor("w_pred", [D + 1, cfg.ncls], f32, kind="ExternalInput")
    wl_in = [nc.dram_tensor(f"wl{l}", [D + 1, D], f32, kind="ExternalInput") for l in range(cfg.nlayer)]
    wr_in = [nc.dram_tensor(f"wr{l}", [D + 1, D], f32, kind="ExternalInput") for l in range(cfg.nlayer)]
    att_in = [nc.dram_tensor(f"att{l}", [P, cfg.wcmax * D], f32, kind="ExternalInput") for l in range(cfg.nlayer)]
    cb_in = [nc.dram_tensor(f"cb{l}", [P, D], f32, kind="ExternalInput") for l in range(cfg.nlayer)]
    padv_in = [nc.dram_tensor(f"padv{l}", [NCHUNK, D], f32, kind="ExternalInput") for l in range(cfg.nlayer)]
    out_dram = nc.dram_tensor("out", [P, NGRP * cfg.ncls], f32, kind="ExternalOutput")
    dbg_h = (nc.dram_tensor("dbg_h", [P, NGRP * (D + 1)], f32, kind="ExternalOutput")
             if cfg.debug else None)

    xl_own = nc.dram_tensor("xl_own", [NL, D], f32)
    if cfg.ncores > 4:  # shared-output collectives need >4 cores
        xl_allg = nc.dram_tensor("xl_allg", [cfg.npad, D], f32, addr_space="Shared")
    else:
        xl_allg = nc.dram_tensor("xl_allg", [cfg.npad, D], f32)
    # gather table: 256B rows, one spare row so the last row's 256B read is safe
    xl_tab = nc.dram_tensor("xl_tab", [cfg.npad + 1, EW], f32)

    with tile.TileContext(nc) as tc:
        with (
            tc.tile_pool(name="const", bufs=1) as cp,
            tc.tile_pool(name="pers", bufs=1) as pp,
            tc.tile_pool(name="work", bufs=3) as wp,
            tc.tile_pool(name="gath", bufs=2) as gp,
            tc.tile_pool(name="edge", bufs=1) as ep,
            tc.tile_pool(name="small", bufs=2) as sp,
            tc.tile_pool(name="ps_mm", bufs=2, space="PSUM") as pmm,
            tc.tile_pool(name="ps_tr", bufs=2, space="PSUM") as ptr,
        ):
            # ---- constants -> SBUF ----
            ident = cp.tile([P, P], f32)
            make_identity(nc, ident[:])
            nj = cfg.inp // P
            wp_sb = cp.tile([P, nj * D], f32)
            nc.sync.dma_start(out=wp_sb[:].rearrange("p (j d) -> p j d", d=D),
                              in_=wp_in[:].rearrange("(j p) d -> p j d", p=P))
            wl_sb = [cp.tile([D + 1, D], f32, name=f"wl_sb{l}") for l in range(cfg.nlayer)]
            wr_sb = [cp.tile([D + 1, D], f32, name=f"wr_sb{l}") for l in range(cfg.nlayer)]
            cb_sb = [cp.tile([P, D], f32, name=f"cb_sb{l}") for l in range(cfg.nlayer)]
            for l in range(cfg.nlayer):
                nc.sync.dma_start(out=wl_sb[l][:], in_=wl_in[l][:])
                nc.sync.dma_start(out=wr_sb[l][:], in_=wr_in[l][:])
                nc.sync.dma_start(out=cb_sb[l][:], in_=cb_in[l][:])
            wpred_sb = cp.tile([D + 1, cfg.ncls], f32)
            nc.sync.dma_start(out=wpred_sb[:], in_=wpred_in[:])
            att_sb = pp.tile([P, cfg.wcmax * D], f32)   # reloaded per layer
            ones_sb = cp.tile([P, 1], f32)
            nc.gpsimd.memset(ones_sb[:], 1.0)

            h_a = pp.tile([P, NGRP * HS], f32, name="h_a")
            h_b = pp.tile([P, NGRP * HS], f32, name="h_b")
            xr_own = pp.tile([P, NGRP * D], f32)

            def h_view(t):  # [P, NGRP, D] data columns
                return t[:].rearrange("p (g s) -> p g s", s=HS)[:, :, :D]

            def ones_col(t):
                return t[:].rearrange("p (g s) -> p g s", s=HS)[:, :, D:HS]

            # ---- P1: h0 = x @ w_proj + b_proj ----
            col_tiles = []
            c0 = 0
            while c0 < NL:
                tw = min(512, NL - c0)
                col_tiles.append((c0, tw))
                c0 += tw
            for (c0, tw) in col_tiles:
                h_acc = pmm.tile([D, 512], f32, tag="h_acc")
                for jj in range(nj):
                    xtile = wp.tile([P, 512], f32, tag="xtile")
                    nc.sync.dma_start(out=xtile[:, :tw], in_=x_t[jj * P:(jj + 1) * P, c0:c0 + tw])
                    nc.tensor.matmul(out=h_acc[:, :tw], lhsT=wp_sb[:, jj * D:(jj + 1) * D],
                                     rhs=xtile[:, :tw], start=(jj == 0), stop=(jj == nj - 1))
                hT_stage = wp.tile([D, 512], f32, tag="hT_stage")
                nc.scalar.copy(out=hT_stage[:, :tw], in_=h_acc[:, :tw])
                for t2 in range(tw // P):
                    chk = (c0 + t2 * P) // P
                    htr = ptr.tile([P, D], f32, tag="htr", bufs=1)
                    nc.tensor.transpose(out=htr[:], in_=hT_stage[:, t2 * P:(t2 + 1) * P],
                                        identity=ident[:D, :D])
                    nc.vector.tensor_copy(out=h_view(h_a)[:, chk, :], in_=htr[:])
            nc.vector.tensor_copy(out=ones_col(h_a)[:, :, 0], in_=ones_sb[:].to_broadcast([P, NGRP]))

            h_cur, h_nxt = h_a, h_b

            # ---- P2: layers ----
            for l in range(cfg.nlayer):
                nc.sync.dma_start(out=att_sb[:], in_=att_in[l][:])
                # (a) xl/xr for owned nodes; xl -> DRAM (+allgather), xr -> SBUF
                xl_dmas = []
                nbatch = 0 if "mm" in ablate else math.ceil(NGRP / 16)
                for b in range(nbatch):
                    chunks = range(b * 16, min((b + 1) * 16, NGRP))
                    hT_chs = {}
                    for chk in chunks:
                        tr = ptr.tile([HS, P], f32, tag="tr")
                        nc.tensor.transpose(
                            out=tr[:], in_=h_cur[:].rearrange("p (g s) -> p g s", s=HS)[:, chk, :],
                            identity=ident[:])
                        hT_ch = wp.tile([HS, P], f32, tag="hT_ch")
                        nc.scalar.copy(out=hT_ch[:], in_=tr[:])
                        hT_chs[chk] = hT_ch
                    for (dst_sb, w_t, to_dram) in ((None, wl_sb[l], True), (xr_own, wr_sb[l], False)):
                        big = pmm.tile([P, 512], f32, tag="big")
                        for i, chk in enumerate(chunks):
                            nc.tensor.matmul(out=big[:, i * D:(i + 1) * D], lhsT=hT_chs[chk][:],
                                             rhs=w_t[:], start=True, stop=True)
                        ncols = len(chunks) * D
                        if to_dram:
                            stage = wp.tile([P, 512], f32, tag="xl_stage")
                            nc.scalar.copy(out=stage[:, :ncols], in_=big[:, :ncols])
                            dma = nc.sync.dma_start(
                                out=xl_own[:].rearrange("(a p) d -> p a d", p=P)[
                                    :, b * 16:b * 16 + len(chunks), :],
                                in_=stage[:, :ncols].rearrange("p (a d) -> p a d", d=D))
                            xl_dmas.append(dma)
                        else:
                            nc.scalar.copy(out=dst_sb[:, b * 16 * D: b * 16 * D + ncols],
                                           in_=big[:, :ncols])
                # (b) allgather xl; expand into 256B-stride table; poison rows
                if "allg" in ablate:
                    pass
                elif cfg.ncores > 1:
                    cc = nc.gpsimd.collective_compute(
                        "AllGather", mybir.AluOpType.bypass,
                        replica_groups=[list(range(cfg.ncores))],
                        ins=[xl_own[:]], outs=[xl_allg[:]])
                    for dma in xl_dmas:
                        add_dep_helper(cc.ins, dma.ins)
                    cpls = []
                    for ci in range(NCHUNK):
                        if "copy" in ablate:
                            break
                        cpl = nc.sync.dma_start(
                            out=xl_tab[ci * CH:(ci + 1) * CH, :D],
                            in_=xl_allg[ci * CH:(ci + 1) * CH, :])
                        add_dep_helper(cpl.ins, cc.ins)
                        cpls.append(cpl)
                else:
                    cpls = []
                    for ci in range(NCHUNK):
                        lo, hi = ci * CH, min((ci + 1) * CH, NL)
                        if lo >= hi:
                            continue
                        cpl = nc.sync.dma_start(out=xl_tab[lo:hi, :D],
                                                in_=xl_own[lo:hi, :])
                        for dma in xl_dmas:
                            add_dep_helper(cpl.ins, dma.ins)
                        cpls.append(cpl)
                if "allg" in ablate or "copy" in ablate:
                    pv = None
                else:
                    pv = nc.sync.dma_start(
                        out=xl_tab[0:cfg.npad, :].rearrange("(a r) e -> a r e", r=CH)[:, CH - 1, :D],
                        in_=padv_in[l][:])
                    for cpl in cpls:
                        add_dep_helper(pv.ins, cpl.ins)

                # (c) edge phase per run
                for (g0, R, rw, colbase) in (() if "edge" in ablate else cfg.runs):
                    cols = R * sum(rw)
                    rcum = [0]
                    for w in rw:
                        rcum.append(rcum[-1] + w)
                    idx_sb = gp.tile([P, 8 * COLS_MAX], i16, tag="idx")
                    nc.sync.dma_start(out=idx_sb[:, :8 * cols],
                                      in_=idx_in[:, 8 * colbase: 8 * (colbase + cols)])
                    xg = gp.tile([P, COLS_MAX * EW], f32, tag="xg")
                    xgq = xg[:].rearrange("p (q e) -> p q e", e=EW)
                    for cidx in range(NCHUNK):
                        W = rw[cidx]
                        if W == 0:
                            continue
                        rs = R * rcum[cidx]
                        ncol = R * W
                        # dma_gather is only reliable up to 1024 idxs/call
                        # (HW-probed); split into <=8-column calls.
                        for off in range(0, ncol, 8):
                            if "gather" in ablate:
                                break
                            nn = min(8, ncol - off)
                            gth = nc.gpsimd.dma_gather(
                                out_ap=xgq[:, rs + off:rs + off + nn, :],
                                in_ap=xl_tab[cidx * CH:(cidx + 1) * CH, :],
                                idxs_ap=idx_sb[:, 8 * (rs + off): 8 * (rs + off + nn)],
                                num_idxs=128 * nn,
                                num_idxs_reg=128 * nn,
                                elem_size=EW,
                            )
                            if pv is not None:
                                add_dep_helper(gth.ins, pv.ins)
                    if "dve" in ablate:
                        continue
                    ex = ep.tile([P, COLS_MAX * H], f32, tag="ex")
                    exq = ex[:].rearrange("p (q h) -> p q h", h=H)
                    # per chunk-region: s = xl + xr (into upper half), leaky,
                    # *att, logit reduce
                    for cidx in range(NCHUNK):
                        W = rw[cidx]
                        if W == 0:
                            continue
                        rs = R * rcum[cidx]
                        ncol = R * W
                        reg = xgq[:, rs:rs + ncol, :]
                        reg_r = reg.rearrange("p (r w) e -> p r w e", w=W)
                        nc.vector.tensor_tensor(
                            out=reg_r[:, :, :, D:EW],
                            in0=reg_r[:, :, :, 0:D],
                            in1=xr_own[:].rearrange("p (g d) -> p g d", d=D)[
                                :, g0:g0 + R, :].unsqueeze(2).to_broadcast([P, R, W, D]),
                            op=mybir.AluOpType.add)
                        s_flat = reg[:, :, D:EW]
                        nc.vector.scalar_tensor_tensor(
                            out=s_flat, in0=s_flat, scalar=NEG_SLOPE, in1=s_flat,
                            op0=mybir.AluOpType.mult, op1=mybir.AluOpType.max)
                        nc.vector.tensor_tensor(
                            out=reg_r[:, :, :, D:EW], in0=reg_r[:, :, :, D:EW],
                            in1=att_sb[:].rearrange("p (w d) -> p w d", d=D)[
                                :, :W, :].unsqueeze(1).to_broadcast([P, R, W, D]),
                            op=mybir.AluOpType.mult)
                        nc.vector.tensor_reduce(
                            out=exq[:, rs:rs + ncol, :],
                            in_=reg[:, :, D:EW].rearrange("p q (hh cc) -> p q hh cc", cc=C),
                            axis=mybir.AxisListType.X, op=mybir.AluOpType.add)
                        nc.scalar.activation(
                            out=ex[:, rs * H:(rs + ncol) * H],
                            in_=ex[:, rs * H:(rs + ncol) * H],
                            func=mybir.ActivationFunctionType.Exp)
                    # denominators: per chunk-region reduce over W, then sum
                    den4 = sp.tile([P, NCHUNK * R * H], f32, tag="den4")
                    nreg = 0
                    for cidx in range(NCHUNK):
                        W = rw[cidx]
                        if W == 0:
                            continue
                        rs = R * rcum[cidx]
                        nc.vector.tensor_reduce(
                            out=den4[:, nreg * R * H:(nreg + 1) * R * H],
                            in_=exq[:, rs:rs + R * W, :].rearrange(
                                "p (r w) h -> p r h w", w=W),
                            axis=mybir.AxisListType.X, op=mybir.AluOpType.add)
                        nreg += 1
                    den = sp.tile([P, R * H], f32, tag="den")
                    nc.vector.tensor_reduce(
                        out=den[:],
                        in_=den4[:, :nreg * R * H].rearrange(
                            "p (n q) -> p q n", n=nreg),
                        axis=mybir.AxisListType.X, op=mybir.AluOpType.add)
                    nc.vector.tensor_scalar_max(den[:], den[:], 1e-16)
                    rec = sp.tile([P, R * H], f32, tag="rec")
                    nc.vector.reciprocal(out=rec[:], in_=den[:])
                    # weighted aggregation: tmp = xl * ex (in place over xl),
                    # numer = sum_w tmp
                    num4 = sp.tile([P, NCHUNK * R * D], f32, tag="num4")
                    nreg = 0
                    for cidx in range(NCHUNK):
                        W = rw[cidx]
                        if W == 0:
                            continue
                        rs = R * rcum[cidx]
                        ncol = R * W
                        reg = xgq[:, rs:rs + ncol, :]
                        nc.vector.tensor_tensor(
                            out=reg[:, :, 0:D].rearrange("p q (hh cc) -> p q hh cc", cc=C),
                            in0=reg[:, :, 0:D].rearrange("p q (hh cc) -> p q hh cc", cc=C),
                            in1=exq[:, rs:rs + ncol, :].unsqueeze(3).to_broadcast(
                                [P, ncol, H, C]),
                            op=mybir.AluOpType.mult)
                        nc.vector.tensor_reduce(
                            out=num4[:, nreg * R * D:(nreg + 1) * R * D],
                            in_=reg[:, :, 0:D].rearrange("p (r w) d -> p r d w", w=W),
                            axis=mybir.AxisListType.X, op=mybir.AluOpType.add)
                        nreg += 1
                    numer = sp.tile([P, R * D], f32, tag="numer")
                    nc.vector.tensor_reduce(
                        out=numer[:],
                        in_=num4[:, :nreg * R * D].rearrange(
                            "p (n q) -> p q n", n=nreg),
                        axis=mybir.AxisListType.X, op=mybir.AluOpType.add)
                    o1 = sp.tile([P, R * D], f32, tag="o1")
                    nc.vector.tensor_tensor(
                        out=o1[:].rearrange("p (r hh cc) -> p r hh cc", r=R, cc=C),
                        in0=numer[:].rearrange("p (r hh cc) -> p r hh cc", r=R, cc=C),
                        in1=rec[:].rearrange("p (r hh) -> p r hh", r=R).unsqueeze(
                            3).to_broadcast([P, R, H, C]),
                        op=mybir.AluOpType.mult)
                    o3 = sp.tile([P, R * D], f32, tag="o3")
                    nc.vector.tensor_tensor(
                        out=o3[:].rearrange("p (r d) -> p r d", d=D),
                        in0=o1[:].rearrange("p (r d) -> p r d", d=D),
                        in1=h_view(h_cur)[:, g0:g0 + R, :],
                        op=mybir.AluOpType.add)
                    nc.vector.tensor_tensor(
                        out=h_view(h_nxt)[:, g0:g0 + R, :],
                        in0=o3[:].rearrange("p (r d) -> p r d", d=D),
                        in1=cb_sb[l][:].unsqueeze(1).to_broadcast([P, R, D]),
                        op=mybir.AluOpType.add)
                nc.vector.tensor_copy(out=ones_col(h_nxt)[:, :, 0],
                                      in_=ones_sb[:].to_broadcast([P, NGRP]))
                h_cur, h_nxt = h_nxt, h_cur

            if dbg_h is not None:
                nc.sync.dma_start(out=dbg_h[:], in_=h_cur[:])

            # ---- P3: out = h2 @ w_pred + b_pred ----
            nbatch = math.ceil(NGRP / 16)
            for b in range(nbatch):
                chunks = range(b * 16, min((b + 1) * 16, NGRP))
                fin = pmm.tile([P, 16 * cfg.ncls], f32, tag="fin", bufs=1)
                for i, chk in enumerate(chunks):
                    tr = ptr.tile([HS, P], f32, tag="tr")
                    nc.tensor.transpose(
                        out=tr[:], in_=h_cur[:].rearrange("p (g s) -> p g s", s=HS)[:, chk, :],
                        identity=ident[:])
                    hT_ch = wp.tile([HS, P], f32, tag="hT_ch")
                    nc.scalar.copy(out=hT_ch[:], in_=tr[:])
                    nc.tensor.matmul(out=fin[:, i * cfg.ncls:(i + 1) * cfg.ncls], lhsT=hT_ch[:],
                                     rhs=wpred_sb[:], start=True, stop=True)
                ncols = len(chunks) * cfg.ncls
                ostage = wp.tile([P, 16 * cfg.ncls], f32, tag="ostage")
                nc.scalar.copy(out=ostage[:, :ncols], in_=fin[:, :ncols])
                nc.sync.dma_start(out=out_dram[:, b * 16 * cfg.ncls: b * 16 * cfg.ncls + ncols],
                                  in_=ostage[:, :ncols])
    nc.finalize()
    return nc


def assemble_output(cfg: Cfg, meta, core_outs):
    """core_outs: list of [128, NGRP*ncls] arrays -> full [n_real, ncls]."""
    full = np.zeros((cfg.npad, cfg.ncls), np.float32)
    for k in range(cfg.ncores):
        o = core_outs[k].reshape(128, cfg.ngrp, cfg.ncls).transpose(1, 0, 2).reshape(cfg.nl, cfg.ncls)
        full[meta["nodes_by_lid"][k]] = o
    return full[:cfg.n_real]


_LAST = {}


def bench(inputs, iters=20) -> dict:
    """Correctness + repeat-execution timing via a hand-rolled PJRT runner
    (no NTFF hook in this environment). Returns output + per-iter seconds."""
    import time

    import jax
    from jax.sharding import Mesh, NamedSharding, PartitionSpec
    from jax.experimental.shard_map import shard_map
    import concourse.mybir as mybir
    from concourse import bass2jax
    from concourse.bass2jax import _bass_exec_p, install_neuronx_cc_hook, partition_id_tensor

    inputs = {k: np.asarray(v) for k, v in inputs.items()}
    cfg, per_core, shared, meta = host_prep(ncores=NCORES, **inputs)
    nc = build_program(cfg)
    in_maps = [{**shared, **pc} for pc in per_core]

    install_neuronx_cc_hook()
    partition_name = nc.partition_id_tensor.name if nc.partition_id_tensor else None
    in_names, out_names, out_avals, zero_outs = [], [], [], []
    for alloc in nc.m.functions[0].allocations:
        if not isinstance(alloc, mybir.MemoryLocationSet):
            continue
        name = alloc.memorylocations[0].name
        if alloc.kind == "ExternalInput":
            if name != partition_name:
                in_names.append(name)
        elif alloc.kind == "ExternalOutput":
            shape = tuple(alloc.tensor_shape)
            dtype = mybir.dt.np(alloc.dtype)
            out_names.append(name)
            out_avals.append(jax.core.ShapedArray(shape, dtype))
            zero_outs.append(np.zeros(shape, dtype))
    n_params = len(in_names)
    n_outs = len(out_avals)
    all_in_names = list(in_names) + list(out_names)
    if partition_name is not None:
        all_in_names.append(partition_name)
    donate = tuple(range(n_params, n_params + n_outs))

    def _body(*args):
        operands = list(args)
        if partition_name is not None:
            operands.append(partition_id_tensor())
        return tuple(_bass_exec_p.bind(
            *operands, out_avals=tuple(out_avals), in_names=tuple(all_in_names),
            out_names=tuple(out_names), lowering_input_output_aliases=(),
            sim_require_finite=True, sim_require_nnan=True, nc=nc))

    devices = jax.devices()[:NCORES]
    mesh = Mesh(np.asarray(devices), ("core",))
    in_specs = (PartitionSpec("core"),) * (n_params + n_outs)
    out_specs = (PartitionSpec("core"),) * n_outs
    sharded = jax.jit(shard_map(_body, mesh=mesh, in_specs=in_specs,
                                out_specs=out_specs, check_rep=False),
                      donate_argnums=donate, keep_unused=True)
    concat_in = [np.concatenate([np.asarray(in_maps[c][n]) for c in range(NCORES)], axis=0)
                 for n in in_names]
    t0 = time.time()
    sh = NamedSharding(mesh, PartitionSpec("core"))
    args_dev = [jax.device_put(a, sh) for a in concat_in]
    jax.block_until_ready(args_dev)
    t_put = time.time() - t0

    def zeros_dev():
        return [jax.device_put(np.zeros((NCORES * z.shape[0], *z.shape[1:]), z.dtype), sh)
                for z in zero_outs]

    t0 = time.time()
    outs = sharded(*args_dev, *zeros_dev())
    jax.block_until_ready(outs)
    t_first = time.time() - t0
    result = [
        {name: np.asarray(outs[i]).reshape(NCORES, *out_avals[i].shape)[c]
         for i, name in enumerate(out_names)} for c in range(NCORES)]

    # warm + timed loop (zeros pre-staged on device; donated per call)
    zsets = [zeros_dev() for _ in range(iters + 2)]
    jax.block_until_ready(zsets)
    r = sharded(*args_dev, *zsets[0])
    jax.block_until_ready(r)
    r = sharded(*args_dev, *zsets[1])
    jax.block_until_ready(r)
    t0 = time.time()
    rs = [sharded(*args_dev, *zsets[2 + i]) for i in range(iters)]
    jax.block_until_ready(rs)
    per_iter = (time.time() - t0) / iters

    out_full = assemble_output(cfg, meta, [result[k]["out"] for k in range(NCORES)])
    return {"out": out_full, "per_iter_s": per_iter, "first_s": t_first,
            "put_s": t_put, "cfg": cfg}


def kernel(**inputs) -> np.ndarray:
    import time

    from concourse.bass_utils import run_bass_kernel_spmd

    inputs = {k: np.asarray(v) for k, v in inputs.items()}
    cfg, per_core, shared, meta = host_prep(ncores=NCORES, **inputs)
    nc = build_program(cfg)
    in_maps = [{**shared, **pc} for pc in per_core]
    res = None
    for attempt in range(2):
        try:
            res = run_bass_kernel_spmd(nc, in_maps, core_ids=list(range(NCORES)))
            break
        except Exception:
            # transient device wedge (NRT_EXEC_UNIT_UNRECOVERABLE) recovers on
            # a fresh attempt once the runtime re-initializes
            if attempt == 1:
                raise
            time.sleep(20)
    assert res is not None
    _LAST["res"] = res
    _LAST["cfg"] = cfg
    outs = [res.results[k]["out"] for k in range(NCORES)]
    return assemble_output(cfg, meta, outs)
